# revision 6
# baseline (speedup 1.0000x reference)
"""Trainium2 Bass kernel for nn_BidirRecurrentModel (moe_routing).

Bidirectional 2-layer GRU, T=256 steps, B=64, H=500, 6 experts routed by
token id (only experts 0..4 are reachable: e = tok // 10000 < 5).

Strategy (8 NeuronCores, SPMD):
  - Hidden dim H padded 500->512, sharded 8 ways (chunk of 64 h-dims per
    core).  Both directions stacked into the M=128 token dim (64 fwd +
    64 bwd), so every matmul runs with a full 128-wide stationary.
  - Each core holds the (r,z,n) weight row-slices for its chunk of all
    3 recurrent weight groups (W_hh0, W_ihL, W_hhL), SBUF-resident, in
    fp32r (full-rate fp32 matmul mode).
  - Layer-0 input gates gi0 depend only on token ids -> precomputed on
    device in a parallel phase (all-expert matmul + one-hot select)
    into a DRAM table, read back 2 rows per step.
  - Per step, one 8-core AllGather carries this step's h0_new chunk and
    the previous step's h1_new chunk (transposed), rebuilding the full
    h^T stationaries on every core.  Layer-1 biases ride inside the gi1
    matmul via one-hot rows appended to the stationary.
  - Expert selection: scalar_tensor_tensor accumulation with per-token
    one-hot columns; gi1+gh1 r,z pre-acts are fused by PSUM
    accumulation, n pre-acts kept separate (GRU needs r * hn).

kernel(**inputs) takes the FULL inputs (as produced by setup_inputs())
and returns (output [T,B,H], h0 [2L,B,H]) matching the reference.
"""

import numpy as np

import concourse.bacc as bacc
import concourse.mybir as mybir
import concourse.tile as tile
from concourse.bass_utils import run_bass_kernel_spmd

F32 = mybir.dt.float32
F32R = mybir.dt.float32r
AF = mybir.ActivationFunctionType
ALU = mybir.AluOpType

NCORES = 8
H = 500
HP = 512            # padded hidden
CH = HP // NCORES   # 64 h-dims per core
NE = 5              # reachable experts
B = 64
M = 2 * B           # fwd + bwd stacked
K = HP              # contraction (pad + bias/one-hot rows)
NRZ = NE * 2 * CH   # 640 r,z cols
NN = NE * CH        # 320 n cols
NW = NRZ + NN       # 960 streamed cols per group
EXPERT_DIV = 10000

# conservative-mode switches (validated fast paths get flipped on)
USE_VT = False      # DVE stream-transpose instead of PE transpose+copy
INPLACE_SEL = True  # in-place scalar_tensor_tensor accumulation


def _bf16_rne(a):
    u = a.view(np.uint32)
    r = ((u >> 16) + (((u >> 15) & 1) & ((u & 0x17FFF) != 0) * 1)).astype(np.uint32) << 16
    return r.view(np.float32)


def round_f32r(a):
    a = np.ascontiguousarray(a, np.float32)
    hi = _bf16_rne(a)
    lo = _bf16_rne((a - hi).astype(np.float32))
    return (hi + lo).astype(np.float32)


def _ktile(img):
    """[K, N] -> [128, K//128 * N] with k-tile k at cols [k*N, (k+1)*N)."""
    Kd, N = img.shape
    nk = Kd // 128
    return np.ascontiguousarray(
        img.reshape(nk, 128, N).transpose(1, 0, 2).reshape(128, nk * N)
    )


def _rz_col(e, j):
    return e * 2 * CH + j          # j in [0, 128): r then z


def _n_col(e, j):
    return NRZ + e * CH + j


def _build_weight_img(Wlist, bias_rz=None, bias_n=None, core=0):
    """Build [K, NW] streamed image for one weight group, rows for this
    core's h-chunk, cols [rz | n] expert-major.  Wlist: [E][1500, 500].
    bias_rz/bias_n: [E][1500] biases placed on one-hot row 500+e for the
    r,z / n column blocks respectively (the GRU applies hidden-side n
    bias inside r*(hn+b), so n biases must ride with their own matmul)."""
    img = np.zeros((K, NW), np.float32)
    c0 = core * CH
    for e in range(NE):
        W = Wlist[e]
        for bi, boff in ((0, 0), (1, 500), (2, 1000)):  # r, z, n
            rows = np.arange(c0, min(c0 + CH, 500))
            wsub = W[boff + rows, :500]                 # [nv, 500]
            if bi < 2:
                cols = _rz_col(e, bi * CH) + np.arange(len(rows))
                bias = bias_rz
            else:
                cols = _n_col(e, 0) + np.arange(len(rows))
                bias = bias_n
            img[:500, cols] = wsub.T
            if bias is not None:
                img[500 + e, cols] = bias[e][boff + rows]
    return round_f32r(img)


def _prep(tokens, emb, W_ih0, W_hh0, b_ih0, b_hh0, W_ihL, W_hhL, b_ihL, b_hhL):
    T = tokens.shape[0]
    tok = np.asarray(tokens)
    eid = (tok // EXPERT_DIV).astype(np.int64)        # [T, B]
    msk = (tok > 0).astype(np.float32)                # [T, B]

    # ---- per-core streamed weight images (k-tiled, fp32r-rounded) ----
    whh0_img, wihl_img, whhl_img, w0_img, bhn_tab = [], [], [], [], []
    biasL = [(b_ihL[e, 0] + b_hhL[e, 0]) for e in range(NE)]
    biasL_i = [b_ihL[e, 0] for e in range(NE)]
    biasL_h = [b_hhL[e, 0] for e in range(NE)]
    bias0 = [(b_ih0[e] + b_hh0[e]) for e in range(NE)]
    bias0_i = [b_ih0[e] for e in range(NE)]
    for c in range(NCORES):
        whh0_img.append(_ktile(_build_weight_img(
            [W_hh0[e] for e in range(NE)], None, None, c)))
        wihl_img.append(_ktile(_build_weight_img(
            [W_ihL[e, 0] for e in range(NE)], biasL, biasL_i, c)))
        whhl_img.append(_ktile(_build_weight_img(
            [W_hhL[e, 0] for e in range(NE)], None, biasL_h, c)))
        # layer-0 hidden-side n bias per token instance: [T*B, CH]
        c0 = c * CH
        nv = min(c0 + CH, 500) - c0
        bh = np.zeros((T * B, CH), np.float32)
        bh[:, :nv] = b_hh0[eid.reshape(-1)][:, 1000 + c0:1000 + c0 + nv]
        bhn_tab.append(bh)
        # parallel-phase image: W_ih0 with combined layer-0 bias on row 500
        img = np.zeros((K, NW), np.float32)
        for e in range(NE):
            for bi, boff in ((0, 0), (1, 500), (2, 1000)):
                rows = np.arange(c0, min(c0 + CH, 500))
                if bi < 2:
                    cols = _rz_col(e, bi * CH) + np.arange(len(rows))
                    bsrc = bias0[e]
                else:
                    cols = _n_col(e, 0) + np.arange(len(rows))
                    bsrc = bias0_i[e]
                img[:500, cols] = W_ih0[e][boff + rows, :500].T
                img[500, cols] = bsrc[boff + rows]
        w0_img.append(_ktile(round_f32r(img)))

    # ---- x_aug^T (shared): emb rows for all T*B instances ----
    x = np.asarray(emb)[tok.reshape(-1)].astype(np.float32)   # [T*B, 500]
    x_aug = np.zeros((T * B, K), np.float32)
    x_aug[:, :500] = x
    x_aug[:, 500] = 1.0
    x_augT = _ktile(round_f32r(np.ascontiguousarray(x_aug.T)))  # [128, 4*T*B]

    # ---- one-hot / mask tables ----
    # parallel selects: [128, (T*B/128) * 5]
    ntile = T * B // 128
    einst = eid.reshape(-1)
    ohp = np.zeros((128, ntile * NE), np.float32)
    for j in range(ntile):
        es = einst[j * 128:(j + 1) * 128]
        ohp[np.arange(128), j * NE + es] = 1.0
    # sequential selects: [128, T*5], rows 0:64 fwd step s, 64:128 bwd
    ohs = np.zeros((128, T * NE), np.float32)
    for s in range(T):
        ohs[np.arange(B), s * NE + eid[s]] = 1.0
        ohs[B + np.arange(B), s * NE + eid[T - 1 - s]] = 1.0
    # one-hot rows DMA'd into X0 k-tile 3 partitions 116..120: [T, 5, 128]
    ohT = np.zeros((T, NE, 128), np.float32)
    for s in range(T):
        ohT[s, eid[s], np.arange(B)] = 1.0
        ohT[s, eid[T - 1 - s], B + np.arange(B)] = 1.0
    # masks [128, T]
    mT = np.zeros((128, T), np.float32)
    for s in range(T):
        mT[:B, s] = msk[s]
        mT[B:, s] = msk[T - 1 - s]

    shared = {
        "x_augT": x_augT, "ohp": ohp, "ohs": ohs, "ohT": ohT, "mT": mT,
        "ident": np.eye(128, dtype=np.float32),
    }
    per_core = [
        {"whh0_img": whh0_img[c], "wihl_img": wihl_img[c],
         "whhl_img": whhl_img[c], "w0_img": w0_img[c], "bhn": bhn_tab[c]}
        for c in range(NCORES)
    ]
    return shared, per_core


def build_program(nc, T):
    nk = K // 128
    NI = T * B          # token instances
    ntile = NI // 128

    # ---- I/O ----
    d_whh0 = nc.dram_tensor("whh0_img", [128, nk * NW], F32R, kind="ExternalInput")
    d_wihl = nc.dram_tensor("wihl_img", [128, nk * NW], F32R, kind="ExternalInput")
    d_whhl = nc.dram_tensor("whhl_img", [128, nk * NW], F32R, kind="ExternalInput")
    d_w0 = nc.dram_tensor("w0_img", [128, nk * NW], F32R, kind="ExternalInput")
    d_xT = nc.dram_tensor("x_augT", [128, nk * NI], F32R, kind="ExternalInput")
    d_ohp = nc.dram_tensor("ohp", [128, ntile * NE], F32, kind="ExternalInput")
    d_ohs = nc.dram_tensor("ohs", [128, T * NE], F32, kind="ExternalInput")
    d_ohT = nc.dram_tensor("ohT", [T, NE, 128], F32R, kind="ExternalInput")
    d_mT = nc.dram_tensor("mT", [128, T], F32, kind="ExternalInput")
    d_ident = nc.dram_tensor("ident", [128, 128], F32, kind="ExternalInput")
    d_bhn = nc.dram_tensor("bhn", [T * B, CH], F32, kind="ExternalInput")
    d_outs = nc.dram_tensor("outs", [T, 128, CH], F32, kind="ExternalOutput")
    d_hfin = nc.dram_tensor("hfin", [2, 128, CH], F32, kind="ExternalOutput")

    gi0_tab = nc.dram_tensor("gi0_tab", [NI, 3 * CH], F32)   # internal

    with tile.TileContext(nc) as tc:
        with (
            tc.tile_pool(name="const", bufs=1) as cp,
            tc.tile_pool(name="seq", bufs=3) as sq,
            tc.tile_pool(name="gates", bufs=3) as gp,
            tc.tile_pool(name="par", bufs=3) as pp,
            tc.tile_pool(name="psA", bufs=1, space="PSUM") as psA,
            tc.tile_pool(name="psB", bufs=1, space="PSUM") as psB,
            tc.tile_pool(name="psC", bufs=1, space="PSUM") as psC,
            tc.tile_pool(name="dram", bufs=3, space="DRAM") as dr,
        ):
            # ---------- persistent SBUF ----------
            w_hh0 = cp.tile([128, nk * NW], F32R, tag="w_hh0")
            w_ihl = cp.tile([128, nk * NW], F32R, tag="w_ihl")
            w_hhl = cp.tile([128, nk * NW], F32R, tag="w_hhl")
            w_0 = cp.tile([128, nk * NW], F32R, tag="w_0")
            ohs_sb = cp.tile([128, T * NE], F32, tag="ohs")
            ohp_sb = cp.tile([128, ntile * NE], F32, tag="ohp")
            mT_sb = cp.tile([128, T], F32, tag="mT")
            X0 = cp.tile([128, nk * M], F32R, tag="X0")
            X1 = cp.tile([128, nk * M], F32R, tag="X1")
            zer = cp.tile([B, M], F32, tag="zer")

            nc.sync.dma_start(w_hh0[:], d_whh0[:])
            nc.sync.dma_start(w_ihl[:], d_wihl[:])
            nc.sync.dma_start(w_hhl[:], d_whhl[:])
            nc.sync.dma_start(w_0[:], d_w0[:])
            nc.sync.dma_start(ohs_sb[:], d_ohs[:])
            nc.sync.dma_start(ohp_sb[:], d_ohp[:])
            nc.sync.dma_start(mT_sb[:], d_mT[:])
            nc.vector.memset(X0[:].bitcast(F32), 0.0)
            nc.vector.memset(X1[:].bitcast(F32), 0.0)
            nc.vector.memset(zer[:], 0.0)

            h0p = gp.tile([128, CH], F32, tag="h0loc")
            h1p = gp.tile([128, CH], F32, tag="h1loc")
            nc.vector.memset(h0p[:], 0.0)
            nc.vector.memset(h1p[:], 0.0)

            # ---------- parallel phase: gi0 table ----------
            for j in range(ntile):
                xs = pp.tile([128, nk * 128], F32R, tag="xs")
                nc.sync.dma_start(
                    xs[:].rearrange("p (k m) -> p k m", k=nk),
                    d_xT[:].rearrange("p (k i) -> p k i", k=nk)[:, :, j * 128:(j + 1) * 128])
                pm = psA.tile([128, 1024], F32, tag="pmA")
                for n0, n1 in ((0, 512), (512, NW)):
                    for k in range(nk):
                        nc.tensor.matmul(
                            pm[:, n0:n1],
                            xs[:, k * 128:(k + 1) * 128],
                            w_0[:, k * NW + n0:k * NW + n1],
                            start=(k == 0), stop=(k == nk - 1))
                gsel = pp.tile([128, 3 * CH], F32, tag="gsel")
                # rz select (5 experts) then n select
                for e in range(NE):
                    oh = ohp_sb[:, j * NE + e:j * NE + e + 1]
                    if e == 0:
                        nc.vector.tensor_scalar_mul(gsel[:, 0:128], pm[:, 0:128], oh)
                        nc.vector.tensor_scalar_mul(
                            gsel[:, 128:192], pm[:, NRZ:NRZ + CH], oh)
                    else:
                        nc.vector.scalar_tensor_tensor(
                            gsel[:, 0:128], pm[:, e * 128:(e + 1) * 128], oh,
                            gsel[:, 0:128], op0=ALU.mult, op1=ALU.add)
                        nc.vector.scalar_tensor_tensor(
                            gsel[:, 128:192], pm[:, NRZ + e * CH:NRZ + (e + 1) * CH],
                            oh, gsel[:, 128:192], op0=ALU.mult, op1=ALU.add)
                nc.sync.dma_start(
                    gi0_tab[j * 128:(j + 1) * 128, :], gsel[:])

            # ---------- helpers ----------
            def mm_group(dst_ps, wimg, stat, accum=False, n_lo=0, n_hi=NW):
                splits, n = [], n_lo
                while n < n_hi:
                    splits.append((n, min(n + 512, n_hi)))
                    n = min(n + 512, n_hi)
                for n0, n1 in splits:
                    for k in range(nk):
                        nc.tensor.matmul(
                            dst_ps[:, n0 - n_lo:n1 - n_lo],
                            stat[:, k * M:(k + 1) * M],
                            wimg[:, k * NW + n0:k * NW + n1],
                            start=(k == 0 and not accum), stop=(k == nk - 1))

            def select(dst, ps, s, col0, width, base=None):
                """dst[:, :width] = sum_e ohs[:,s*5+e] * ps[:, col0_e : +width]
                (+ base as init addend on e==0 if given)."""
                for e in range(NE):
                    oh = ohs_sb[:, s * NE + e:s * NE + e + 1]
                    src = ps[:, col0(e):col0(e) + width]
                    if e == 0:
                        if base is None:
                            nc.vector.tensor_scalar_mul(dst, src, oh)
                        else:
                            nc.vector.scalar_tensor_tensor(
                                dst, src, oh, base, op0=ALU.mult, op1=ALU.add)
                    else:
                        nc.vector.scalar_tensor_tensor(
                            dst, src, oh, dst, op0=ALU.mult, op1=ALU.add)

            def gates(s, rz_sel, inn, hn_sel, hprev, pool_tag):
                """returns h_new [128, CH] token-layout."""
                mcol = mT_sb[:, s:s + 1]
                act = gp.tile([128, 2 * CH], F32, tag=pool_tag + "act")
                nc.scalar.activation(act[:, 0:128], rz_sel, AF.Sigmoid)
                rhn = gp.tile([128, CH], F32, tag=pool_tag + "rhn")
                nc.gpsimd.tensor_mul(rhn[:], act[:, 0:CH], hn_sel)
                nc.gpsimd.tensor_add(rhn[:], rhn[:], inn)
                nn_t = gp.tile([128, CH], F32, tag=pool_tag + "nn")
                nc.scalar.activation(nn_t[:], rhn[:], AF.Tanh)
                t_t = gp.tile([128, CH], F32, tag=pool_tag + "t")
                nc.gpsimd.tensor_sub(t_t[:], nn_t[:], hprev)
                u_t = gp.tile([128, CH], F32, tag=pool_tag + "u")
                nc.gpsimd.tensor_mul(u_t[:], act[:, CH:2 * CH], t_t[:])
                nc.gpsimd.tensor_sub(t_t[:], t_t[:], u_t[:])
                hnew = gp.tile([128, CH], F32, tag=pool_tag + "hnew")
                nc.vector.scalar_tensor_tensor(
                    hnew[:], t_t[:], mcol, hprev, op0=ALU.mult, op1=ALU.add)
                return hnew

            def transpose_out(hnew, agin_dr, row0, pool_tag):
                """[128, CH] -> transposed [CH, 128] -> DRAM agin rows."""
                pt = psC.tile([CH, 128], F32, tag="pt")
                nc.tensor.transpose(pt[:], hnew[:], ident_sb[:])
                tt = gp.tile([CH, 128], F32R, tag=pool_tag + "tt")
                nc.scalar.copy(tt[:], pt[:])
                nc.sync.dma_start(agin_dr[row0:row0 + CH, :], tt[:])

            ident_sb = cp.tile([128, 128], F32, tag="ident")
            nc.sync.dma_start(ident_sb[:], d_ident[:])

            # ---------- sequential phase ----------
            agin_tiles = {}
            agout_tiles = {}

            def readback(ago):
                agv = ago[:].rearrange(
                    "(ko ph l c) m -> l ph c ko m", ko=nk, ph=2, l=2, c=CH)
                for ph in range(2):
                    nc.sync.dma_start(
                        X0[ph * CH:(ph + 1) * CH, :]
                        .rearrange("c (k m) -> c k m", k=nk), agv[0, ph])
                    nc.sync.dma_start(
                        X1[ph * CH:(ph + 1) * CH, :]
                        .rearrange("c (k m) -> c k m", k=nk), agv[1, ph])

            def do_L1(s):
                """L1 for step s: needs X0=h0_new(s), X1=h1_new(s-1),
                one-hot rows of step s already in X0."""
                pmB = psB.tile([128, 1024], F32, tag="pmB")
                pmC = psC.tile([128, 512], F32, tag="pmC")
                mm_group(pmB, w_ihl, X0)                      # gi1 (all cols)
                mm_group(pmB, w_hhl, X1, accum=True, n_hi=NRZ)  # gh1 rz accum
                mm_group(pmC, w_hhl, X1, n_lo=NRZ)            # gh1 n -> C[0:320]
                rz1 = gp.tile([128, 128], F32, tag="rz1")
                select(rz1[:], pmB, s, lambda e: e * 2 * CH, 128)
                in1 = gp.tile([128, CH], F32, tag="in1")
                select(in1[:], pmB, s, lambda e: NRZ + e * CH, CH)
                hn1 = gp.tile([128, CH], F32, tag="hn1")
                select(hn1[:], pmC, s, lambda e: e * CH, CH)
                return gates(s, rz1[:], in1[:], hn1[:], h1p[:], "L1")

            for s in range(T):
                # -- readback AG(s-1) --
                if s > 0:
                    ago = agout_tiles.pop(s - 1)
                    readback(ago)
                    # one-hot rows for step s-1 (bias rows for L1(s-1))
                    nc.sync.dma_start(X0[116:121, 3 * M:4 * M], d_ohT[s - 1])
                    nc.sync.dma_start(X1[116:121, 3 * M:4 * M], d_ohT[s - 1])

                # -- L0(s) --
                pmA = psA.tile([128, 1024], F32, tag="pmA")
                mm_group(pmA, w_hh0, X0)
                gi0 = sq.tile([128, 4 * CH], F32, tag="gi0")
                nc.sync.dma_start(gi0[0:B, 0:192], gi0_tab[s * B:(s + 1) * B, :])
                nc.sync.dma_start(
                    gi0[B:128, 0:192], gi0_tab[(T - 1 - s) * B:(T - s) * B, :])
                nc.sync.dma_start(gi0[0:B, 192:256], d_bhn[s * B:(s + 1) * B, :])
                nc.sync.dma_start(
                    gi0[B:128, 192:256], d_bhn[(T - 1 - s) * B:(T - s) * B, :])
                rz0 = gp.tile([128, 128], F32, tag="rz0")
                select(rz0[:], pmA, s, lambda e: e * 2 * CH, 128,
                       base=gi0[:, 0:128])
                hn0 = gp.tile([128, CH], F32, tag="hn0")
                select(hn0[:], pmA, s, lambda e: NRZ + e * CH, CH,
                       base=gi0[:, 192:256])
                h0n = gates(s, rz0[:], gi0[:, 128:192], hn0[:], h0p[:], "L0")

                agin = dr.tile([M, 128], F32R, tag="agin")
                agin_tiles[s] = agin
                transpose_out(h0n, agin, 0, "T0")

                # -- L1(s-1) --
                if s > 0:
                    h1n = do_L1(s - 1)
                    transpose_out(h1n, agin, B, "T1")
                    nc.sync.dma_start(d_outs[s - 1], h1n[:])
                    h1p = h1n
                else:
                    nc.sync.dma_start(agin[B:M, :].bitcast(F32), zer[:])
                h0p = h0n

                # -- AllGather(s) --
                agout = dr.tile([NCORES * M, 128], F32R, tag="agout")
                agout_tiles[s] = agout
                nc.gpsimd.collective_compute(
                    "AllGather", ALU.bypass,
                    replica_groups=[list(range(NCORES))],
                    ins=[agin.opt()], outs=[agout.opt()])

            # ---------- epilogue: L1(T-1) ----------
            ago = agout_tiles.pop(T - 1)
            readback(ago)
            nc.sync.dma_start(X0[116:121, 3 * M:4 * M], d_ohT[T - 1])
            nc.sync.dma_start(X1[116:121, 3 * M:4 * M], d_ohT[T - 1])
            h1n = do_L1(T - 1)
            nc.sync.dma_start(d_outs[T - 1], h1n[:])
            nc.sync.dma_start(d_hfin[0], h0p[:])
            nc.sync.dma_start(d_hfin[1], h1n[:])
    return nc


_CACHE = {}


def kernel(tokens, emb, W_ih0, W_hh0, b_ih0, b_hh0, W_ihL, W_hhL, b_ihL, b_hhL,
           _trace=False):
    tokens = np.asarray(tokens)
    emb = np.asarray(emb, np.float32)
    T = tokens.shape[0]

    shared, per_core = _prep(tokens, emb,
                             np.asarray(W_ih0, np.float32),
                             np.asarray(W_hh0, np.float32),
                             np.asarray(b_ih0, np.float32),
                             np.asarray(b_hh0, np.float32),
                             np.asarray(W_ihL, np.float32),
                             np.asarray(W_hhL, np.float32),
                             np.asarray(b_ihL, np.float32),
                             np.asarray(b_hhL, np.float32))

    if T not in _CACHE:
        nc = bacc.Bacc(None, num_devices=NCORES)
        build_program(nc, T)
        nc.finalize()
        _CACHE[T] = nc
    nc = _CACHE[T]

    in_maps = [{**shared, **per_core[c]} for c in range(NCORES)]
    res = run_bass_kernel_spmd(nc, in_maps, list(range(NCORES)), trace=_trace)

    outs = [res.results[c]["outs"] for c in range(NCORES)]   # [T,128,CH]
    hfin = [res.results[c]["hfin"] for c in range(NCORES)]   # [2,128,CH]

    outs_full = np.concatenate(outs, axis=2)                 # [T,128,512]
    outs_f = outs_full[:, 0:B, :H]                           # [T,B,H]
    outs_b = outs_full[:, B:128, :H]
    output = (outs_f[::-1] + outs_b) * 0.5

    hf = np.concatenate(hfin, axis=2)                        # [2,128,512]
    h0 = np.stack([hf[0, 0:B, :H], hf[1, 0:B, :H],
                   hf[0, B:128, :H], hf[1, B:128, :H]])      # [4,B,H]
    if _trace:
        return (output.astype(np.float32), h0.astype(np.float32)), res
    return output.astype(np.float32), h0.astype(np.float32)


# revision 10
# speedup vs baseline: 10.7130x; 10.7130x over previous
"""Trainium2 Bass kernel for nn_BidirRecurrentModel (moe_routing).

Bidirectional 2-layer GRU, T=256 steps, B=64, H=500, 6 experts routed by
token id (only experts 0..4 are reachable: e = tok // 10000 < 5).

Strategy (8 NeuronCores, SPMD):
  - Hidden dim H padded 500->512, sharded 8 ways (chunk of 64 h-dims per
    core).  Both directions stacked into the M=128 token dim (64 fwd +
    64 bwd), so every matmul runs with a full 128-wide stationary.
  - Each core holds the (r,z,n) weight row-slices for its chunk of all
    3 recurrent weight groups (W_hh0, W_ihL, W_hhL), SBUF-resident, in
    fp32r (full-rate fp32 matmul mode).
  - Layer-0 input gates gi0 depend only on token ids -> precomputed on
    device in a parallel phase (all-expert matmul + one-hot select)
    into a DRAM table, read back 2 rows per step.
  - Per step, one 8-core AllGather carries this step's h0_new chunk and
    the previous step's h1_new chunk (transposed), rebuilding the full
    h^T stationaries on every core.  Layer-1 biases ride inside the gi1
    matmul via one-hot rows appended to the stationary.
  - Expert selection: scalar_tensor_tensor accumulation with per-token
    one-hot columns; gi1+gh1 r,z pre-acts are fused by PSUM
    accumulation, n pre-acts kept separate (GRU needs r * hn).

kernel(**inputs) takes the FULL inputs (as produced by setup_inputs())
and returns (output [T,B,H], h0 [2L,B,H]) matching the reference.
"""

import numpy as np

import concourse.bacc as bacc
import concourse.mybir as mybir
import concourse.tile as tile
from concourse.bass_utils import run_bass_kernel_spmd

F32 = mybir.dt.float32
F32R = mybir.dt.float32r
AF = mybir.ActivationFunctionType
ALU = mybir.AluOpType

NCORES = 8
H = 500
HP = 512            # padded hidden
CH = HP // NCORES   # 64 h-dims per core
NE = 5              # reachable experts
B = 64
M = 2 * B           # fwd + bwd stacked
K = HP              # contraction (pad + bias/one-hot rows)
NRZ = NE * 2 * CH   # 640 r,z cols
NN = NE * CH        # 320 n cols
NW = NRZ + NN       # 960 streamed cols per group
EXPERT_DIV = 10000

# conservative-mode switches (validated fast paths get flipped on)
USE_VT = False      # DVE stream-transpose instead of PE transpose+copy
INPLACE_SEL = True  # in-place scalar_tensor_tensor accumulation


def _bf16_rne(a):
    u = a.view(np.uint32)
    r = ((u >> 16) + (((u >> 15) & 1) & ((u & 0x17FFF) != 0) * 1)).astype(np.uint32) << 16
    return r.view(np.float32)


def round_f32r(a):
    a = np.ascontiguousarray(a, np.float32)
    hi = _bf16_rne(a)
    lo = _bf16_rne((a - hi).astype(np.float32))
    return (hi + lo).astype(np.float32)


def _ktile(img):
    """[K, N] -> [128, K//128 * N] with k-tile k at cols [k*N, (k+1)*N)."""
    Kd, N = img.shape
    nk = Kd // 128
    return np.ascontiguousarray(
        img.reshape(nk, 128, N).transpose(1, 0, 2).reshape(128, nk * N)
    )


def _rz_col(e, j):
    return e * 2 * CH + j          # j in [0, 128): r then z


def _n_col(e, j):
    return NRZ + e * CH + j


def _build_weight_img(Wlist, bias_rz=None, bias_n=None, core=0):
    """Build [K, NW] streamed image for one weight group, rows for this
    core's h-chunk, cols [rz | n] expert-major.  Wlist: [E][1500, 500].
    bias_rz/bias_n: [E][1500] biases placed on one-hot row 500+e for the
    r,z / n column blocks respectively (the GRU applies hidden-side n
    bias inside r*(hn+b), so n biases must ride with their own matmul)."""
    img = np.zeros((K, NW), np.float32)
    c0 = core * CH
    for e in range(NE):
        W = Wlist[e]
        for bi, boff in ((0, 0), (1, 500), (2, 1000)):  # r, z, n
            rows = np.arange(c0, min(c0 + CH, 500))
            wsub = W[boff + rows, :500]                 # [nv, 500]
            if bi < 2:
                cols = _rz_col(e, bi * CH) + np.arange(len(rows))
                bias = bias_rz
            else:
                cols = _n_col(e, 0) + np.arange(len(rows))
                bias = bias_n
            img[:500, cols] = wsub.T
            if bias is not None:
                img[500 + e, cols] = bias[e][boff + rows]
    return round_f32r(img)


def _prep(tokens, emb, W_ih0, W_hh0, b_ih0, b_hh0, W_ihL, W_hhL, b_ihL, b_hhL):
    T = tokens.shape[0]
    tok = np.asarray(tokens)
    eid = (tok // EXPERT_DIV).astype(np.int64)        # [T, B]
    msk = (tok > 0).astype(np.float32)                # [T, B]

    # ---- per-core streamed weight images (k-tiled, fp32r-rounded) ----
    whh0_img, wihl_img, whhl_img, w0_img, bhn_tab = [], [], [], [], []
    biasL = [(b_ihL[e, 0] + b_hhL[e, 0]) for e in range(NE)]
    biasL_i = [b_ihL[e, 0] for e in range(NE)]
    biasL_h = [b_hhL[e, 0] for e in range(NE)]
    bias0 = [(b_ih0[e] + b_hh0[e]) for e in range(NE)]
    bias0_i = [b_ih0[e] for e in range(NE)]
    for c in range(NCORES):
        whh0_img.append(_ktile(_build_weight_img(
            [W_hh0[e] for e in range(NE)], None, None, c)))
        wihl_img.append(_ktile(_build_weight_img(
            [W_ihL[e, 0] for e in range(NE)], biasL, biasL_i, c)))
        whhl_img.append(_ktile(_build_weight_img(
            [W_hhL[e, 0] for e in range(NE)], None, biasL_h, c)))
        # layer-0 hidden-side n bias per token instance: [T*B, CH]
        c0 = c * CH
        nv = min(c0 + CH, 500) - c0
        bh = np.zeros((T * B, CH), np.float32)
        bh[:, :nv] = b_hh0[eid.reshape(-1)][:, 1000 + c0:1000 + c0 + nv]
        bhn_tab.append(bh)
        # parallel-phase image: W_ih0 with combined layer-0 bias on row 500
        img = np.zeros((K, NW), np.float32)
        for e in range(NE):
            for bi, boff in ((0, 0), (1, 500), (2, 1000)):
                rows = np.arange(c0, min(c0 + CH, 500))
                if bi < 2:
                    cols = _rz_col(e, bi * CH) + np.arange(len(rows))
                    bsrc = bias0[e]
                else:
                    cols = _n_col(e, 0) + np.arange(len(rows))
                    bsrc = bias0_i[e]
                img[:500, cols] = W_ih0[e][boff + rows, :500].T
                img[500, cols] = bsrc[boff + rows]
        w0_img.append(_ktile(round_f32r(img)))

    # ---- x_aug^T (shared): emb rows for all T*B instances ----
    x = np.asarray(emb)[tok.reshape(-1)].astype(np.float32)   # [T*B, 500]
    x_aug = np.zeros((T * B, K), np.float32)
    x_aug[:, :500] = x
    x_aug[:, 500] = 1.0
    x_augT = _ktile(round_f32r(np.ascontiguousarray(x_aug.T)))  # [128, 4*T*B]

    # ---- one-hot / mask tables ----
    # parallel selects: [128, (T*B/128) * 5]
    ntile = T * B // 128
    einst = eid.reshape(-1)
    ohp = np.zeros((128, ntile * NE), np.float32)
    for j in range(ntile):
        es = einst[j * 128:(j + 1) * 128]
        ohp[np.arange(128), j * NE + es] = 1.0
    # sequential selects: [128, T*5], rows 0:64 fwd step s, 64:128 bwd
    ohs = np.zeros((128, T * NE), np.float32)
    for s in range(T):
        ohs[np.arange(B), s * NE + eid[s]] = 1.0
        ohs[B + np.arange(B), s * NE + eid[T - 1 - s]] = 1.0
    # one-hot rows DMA'd into X0 k-tile 3 partitions 116..120: [T, 5, 128]
    ohT = np.zeros((T, NE, 128), np.float32)
    for s in range(T):
        ohT[s, eid[s], np.arange(B)] = 1.0
        ohT[s, eid[T - 1 - s], B + np.arange(B)] = 1.0
    # masks [128, T]
    mT = np.zeros((128, T), np.float32)
    for s in range(T):
        mT[:B, s] = msk[s]
        mT[B:, s] = msk[T - 1 - s]

    shared = {
        "x_augT": x_augT, "ohp": ohp, "ohs": ohs, "ohT": ohT, "mT": mT,
        "ident": np.eye(128, dtype=np.float32),
    }
    per_core = [
        {"whh0_img": whh0_img[c], "wihl_img": wihl_img[c],
         "whhl_img": whhl_img[c], "w0_img": w0_img[c], "bhn": bhn_tab[c]}
        for c in range(NCORES)
    ]
    return shared, per_core


def build_program(nc, T):
    nk = K // 128
    NI = T * B          # token instances
    ntile = NI // 128

    # ---- I/O ----
    d_whh0 = nc.dram_tensor("whh0_img", [128, nk * NW], F32R, kind="ExternalInput")
    d_wihl = nc.dram_tensor("wihl_img", [128, nk * NW], F32R, kind="ExternalInput")
    d_whhl = nc.dram_tensor("whhl_img", [128, nk * NW], F32R, kind="ExternalInput")
    d_w0 = nc.dram_tensor("w0_img", [128, nk * NW], F32R, kind="ExternalInput")
    d_xT = nc.dram_tensor("x_augT", [128, nk * NI], F32R, kind="ExternalInput")
    d_ohp = nc.dram_tensor("ohp", [128, ntile * NE], F32, kind="ExternalInput")
    d_ohs = nc.dram_tensor("ohs", [128, T * NE], F32, kind="ExternalInput")
    d_ohT = nc.dram_tensor("ohT", [T, NE, 128], F32R, kind="ExternalInput")
    d_mT = nc.dram_tensor("mT", [128, T], F32, kind="ExternalInput")
    d_ident = nc.dram_tensor("ident", [128, 128], F32, kind="ExternalInput")
    d_bhn = nc.dram_tensor("bhn", [T * B, CH], F32, kind="ExternalInput")
    d_outs = nc.dram_tensor("outs", [T, 128, CH], F32, kind="ExternalOutput")
    d_hfin = nc.dram_tensor("hfin", [2, 128, CH], F32, kind="ExternalOutput")

    gi0_tab = nc.dram_tensor("gi0_tab", [NI, 3 * CH], F32)   # internal

    with tile.TileContext(nc) as tc:
        with (
            tc.tile_pool(name="const", bufs=1) as cp,
            tc.tile_pool(name="seq", bufs=3) as sq,
            tc.tile_pool(name="gates", bufs=3) as gp,
            tc.tile_pool(name="par", bufs=3) as pp,
            tc.tile_pool(name="psA", bufs=1, space="PSUM") as psA,
            tc.tile_pool(name="psB", bufs=1, space="PSUM") as psB,
            tc.tile_pool(name="psC", bufs=1, space="PSUM") as psC,
            tc.tile_pool(name="dram", bufs=3, space="DRAM") as dr,
        ):
            # ---------- persistent SBUF ----------
            w_hh0 = cp.tile([128, nk * NW], F32R, tag="w_hh0")
            w_ihl = cp.tile([128, nk * NW], F32R, tag="w_ihl")
            w_hhl = cp.tile([128, nk * NW], F32R, tag="w_hhl")
            w_0 = cp.tile([128, nk * NW], F32R, tag="w_0")
            ohs_sb = cp.tile([128, T * NE], F32, tag="ohs")
            ohp_sb = cp.tile([128, ntile * NE], F32, tag="ohp")
            mT_sb = cp.tile([128, T], F32, tag="mT")
            X0 = cp.tile([128, nk * M], F32R, tag="X0")
            X1 = cp.tile([128, nk * M], F32R, tag="X1")
            zer = cp.tile([B, M], F32, tag="zer")

            nc.sync.dma_start(w_hh0[:], d_whh0[:])
            nc.sync.dma_start(w_ihl[:], d_wihl[:])
            nc.sync.dma_start(w_hhl[:], d_whhl[:])
            nc.sync.dma_start(w_0[:], d_w0[:])
            nc.sync.dma_start(ohs_sb[:], d_ohs[:])
            nc.sync.dma_start(ohp_sb[:], d_ohp[:])
            nc.sync.dma_start(mT_sb[:], d_mT[:])
            nc.vector.memset(X0[:].bitcast(F32), 0.0)
            nc.vector.memset(X1[:].bitcast(F32), 0.0)
            nc.vector.memset(zer[:], 0.0)

            ident_sb = cp.tile([128, 128], F32, tag="ident")
            nc.sync.dma_start(ident_sb[:], d_ident[:])

            # ---------- parallel phase: gi0 table ----------
            for j in range(ntile):
                xs = pp.tile([128, nk * 128], F32R, tag="xs")
                nc.sync.dma_start(
                    xs[:].rearrange("p (k m) -> p k m", k=nk),
                    d_xT[:].rearrange("p (k i) -> p k i", k=nk)[:, :, j * 128:(j + 1) * 128])
                pm = psA.tile([128, 1024], F32, tag="pmA")
                for n0, n1 in ((0, 512), (512, NW)):
                    for k in range(nk):
                        nc.tensor.matmul(
                            pm[:, n0:n1],
                            xs[:, k * 128:(k + 1) * 128],
                            w_0[:, k * NW + n0:k * NW + n1],
                            start=(k == 0), stop=(k == nk - 1))
                gsel = pp.tile([128, 3 * CH], F32, tag="gsel")
                # rz select (5 experts) then n select
                for e in range(NE):
                    oh = ohp_sb[:, j * NE + e:j * NE + e + 1]
                    if e == 0:
                        nc.vector.tensor_scalar_mul(gsel[:, 0:128], pm[:, 0:128], oh)
                        nc.vector.tensor_scalar_mul(
                            gsel[:, 128:192], pm[:, NRZ:NRZ + CH], oh)
                    else:
                        nc.vector.scalar_tensor_tensor(
                            gsel[:, 0:128], pm[:, e * 128:(e + 1) * 128], oh,
                            gsel[:, 0:128], op0=ALU.mult, op1=ALU.add)
                        nc.vector.scalar_tensor_tensor(
                            gsel[:, 128:192], pm[:, NRZ + e * CH:NRZ + (e + 1) * CH],
                            oh, gsel[:, 128:192], op0=ALU.mult, op1=ALU.add)
                nc.sync.dma_start(
                    gi0_tab[j * 128:(j + 1) * 128, :], gsel[:])

            # ---------- helpers ----------
            # PSUM bank layout (one [128, 3072] tile = 6 banks):
            #   A (gh0)          cols    0: 960  [rz 640 | n 320]
            #   B (gi1 + gh1rz)  cols 1024:1984  [rz 640 | n 320]
            #   C (gh1 n)        cols 2048:2368
            #   Tr (transpose)   cols 2560:2688
            A0, B0, C0, T0 = 0, 1024, 2048, 2560

            def mm_seq(ps, X0t, X1t):
                # PE order: C first (frees hn1 select early), then A, then B
                for n0, n1, img, stat, acc in (
                        (C0, C0 + 320, w_hhl, X1t, "n"),      # gh1 n
                        (A0, A0 + 512, w_hh0, X0t, None),
                        (A0 + 512, A0 + 960, w_hh0, X0t, None),
                        (B0, B0 + 512, w_ihl, X0t, None),
                        (B0 + 512, B0 + 960, w_ihl, X0t, None),
                        (B0, B0 + 512, w_hhl, X1t, "acc"),     # gh1 rz accum
                        (B0 + 512, B0 + 640, w_hhl, X1t, "acc")):
                    base = {None: A0 if n0 < 1024 else B0, "acc": B0, "n": C0 - NRZ}[acc]
                    woff = (n0 - base) if acc != "n" else NRZ + (n0 - C0)
                    for k in range(nk):
                        nc.tensor.matmul(
                            ps[:, n0:n1],
                            stat[:, k * M:(k + 1) * M],
                            img[:, k * NW + woff:k * NW + woff + (n1 - n0)],
                            start=(k == 0 and acc != "acc"), stop=(k == nk - 1))

            def sel5(dst, ps, scol, col0, width, base=None):
                for e in range(NE):
                    oh = ohs_sb[:, scol * NE + e:scol * NE + e + 1]
                    src = ps[:, col0 + e * width:col0 + (e + 1) * width]
                    if e == 0 and base is None:
                        nc.vector.tensor_scalar_mul(dst, src, oh)
                    else:
                        nc.vector.scalar_tensor_tensor(
                            dst, src, oh, dst if e else base,
                            op0=ALU.mult, op1=ALU.add)

            def readback(ago):
                # agout rows = (rank, layer, ch); h-dim d = rank*CH + ch.
                # X k-tile k partition p <- d = k*128+p: ranks 2k, 2k+1.
                # Each piece is a fully contiguous 32KB block in DRAM.
                for k in range(nk):
                    for r2 in range(2):
                        row = (2 * k + r2) * 128
                        nc.sync.dma_start(
                            X0[r2 * CH:(r2 + 1) * CH, k * M:(k + 1) * M],
                            ago[row:row + CH, :])
                        nc.sync.dma_start(
                            X1[r2 * CH:(r2 + 1) * CH, k * M:(k + 1) * M],
                            ago[row + CH:row + 2 * CH, :])

            # ---------- sequential phase ----------
            agout_tiles = {}
            hC = gp.tile([128, 128], F32, tag="hC")   # [h0 | h1] local
            nc.vector.memset(hC[:], 0.0)

            for s in range(T):
                if s > 0:
                    ago = agout_tiles.pop(s - 1)
                    readback(ago)
                    nc.sync.dma_start(X0[116:121, 3 * M:4 * M], d_ohT[s - 1])
                    nc.sync.dma_start(X1[116:121, 3 * M:4 * M], d_ohT[s - 1])
                sc1 = max(s - 1, 0)   # L1 lane step index (dummy at s=0)

                ps = psA.tile([128, 3072], F32, tag="ps")
                mm_seq(ps, X0, X1)

                gi0 = sq.tile([128, 128], F32, tag="gi0")
                nc.sync.dma_start(gi0[0:B, :], gi0_tab[s * B:(s + 1) * B, 0:128])
                nc.sync.dma_start(
                    gi0[B:128, :], gi0_tab[(T - 1 - s) * B:(T - s) * B, 0:128])
                bhn = sq.tile([128, CH], F32, tag="bhn")
                nc.sync.dma_start(bhn[0:B, :], d_bhn[s * B:(s + 1) * B, :])
                nc.sync.dma_start(
                    bhn[B:128, :], d_bhn[(T - 1 - s) * B:(T - s) * B, :])

                # selects -> rzC [L0 | L1], nC [inn0 | hn0 | hn1 | inn1]
                rzC = gp.tile([128, 256], F32, tag="rzC")
                nC = gp.tile([128, 256], F32, tag="nC")
                nc.sync.dma_start(
                    nC[0:B, 0:CH], gi0_tab[s * B:(s + 1) * B, 128:192])
                nc.sync.dma_start(
                    nC[B:128, 0:CH],
                    gi0_tab[(T - 1 - s) * B:(T - s) * B, 128:192])
                sel5(nC[:, 192:256], ps, sc1, C0, CH)                  # hn1
                sel5(rzC[:, 0:128], ps, s, A0, 128, base=gi0[:])       # rz0
                sel5(nC[:, 128:192], ps, s, A0 + NRZ, CH, base=bhn[:])   # hn0
                sel5(rzC[:, 128:256], ps, sc1, B0, 128)                # rz1
                sel5(nC[:, 64:128], ps, sc1, B0 + NRZ, CH)             # inn1

                # combined gates ([128,128] ops, lanes L0|L1)
                act = gp.tile([128, 256], F32, tag="act")
                nc.scalar.activation(act[:], rzC[:], AF.Sigmoid)
                lrz = act[:].rearrange("p (l rz c) -> p l rz c", l=2, rz=2)
                r_ap, z_ap = lrz[:, :, 0, :], lrz[:, :, 1, :]
                rhn = gp.tile([128, 128], F32, tag="rhn")
                rhn2 = rhn[:].rearrange("p (l c) -> p l c", l=2)
                nc.gpsimd.tensor_tensor(
                    rhn2, r_ap,
                    nC[:, 128:256].rearrange("p (l c) -> p l c", l=2),
                    op=ALU.mult)
                nc.gpsimd.tensor_tensor(rhn[:], rhn[:], nC[:, 0:128],
                                        op=ALU.add)
                nt = gp.tile([128, 128], F32, tag="nt")
                nc.scalar.activation(nt[:], rhn[:], AF.Tanh)
                tt_ = gp.tile([128, 128], F32, tag="tt_")
                nc.gpsimd.tensor_tensor(tt_[:], nt[:], hC[:], op=ALU.subtract)
                ut = gp.tile([128, 128], F32, tag="ut")
                nc.gpsimd.tensor_tensor(
                    ut[:].rearrange("p (l c) -> p l c", l=2), z_ap,
                    tt_[:].rearrange("p (l c) -> p l c", l=2), op=ALU.mult)
                nc.gpsimd.tensor_tensor(tt_[:], tt_[:], ut[:],
                                        op=ALU.subtract)
                hN = gp.tile([128, 128], F32, tag="hC")
                nc.vector.scalar_tensor_tensor(
                    hN[:, 0:CH], tt_[:, 0:CH], mT_sb[:, s:s + 1],
                    hC[:, 0:CH], op0=ALU.mult, op1=ALU.add)
                nc.vector.scalar_tensor_tensor(
                    hN[:, CH:128], tt_[:, CH:128], mT_sb[:, sc1:sc1 + 1],
                    hC[:, CH:128], op0=ALU.mult, op1=ALU.add)
                if s > 0:
                    nc.sync.dma_start(d_outs[s - 1], hN[:, CH:128])
                hC = hN

                # transpose both halves at once -> agin
                nc.tensor.transpose(ps[:, T0:T0 + 128], hN[:], ident_sb[:])
                ttile = gp.tile([128, 128], F32R, tag="ttile")
                nc.scalar.copy(ttile[:], ps[:, T0:T0 + 128])
                agin = dr.tile([M, 128], F32R, tag="agin")
                nc.sync.dma_start(agin[:], ttile[:])

                agout = dr.tile([NCORES * M, 128], F32R, tag="agout")
                agout_tiles[s] = agout
                nc.gpsimd.collective_compute(
                    "AllGather", ALU.bypass,
                    replica_groups=[list(range(NCORES))],
                    ins=[agin.opt()], outs=[agout.opt()])

            # ---------- epilogue: L1(T-1) ----------
            ago = agout_tiles.pop(T - 1)
            readback(ago)
            nc.sync.dma_start(X0[116:121, 3 * M:4 * M], d_ohT[T - 1])
            nc.sync.dma_start(X1[116:121, 3 * M:4 * M], d_ohT[T - 1])
            ps = psA.tile([128, 3072], F32, tag="ps")
            mm_seq(ps, X0, X1)
            se = T - 1
            rz1 = gp.tile([128, 128], F32, tag="rzC")
            sel5(rz1[:], ps, se, B0, 128)
            in1 = gp.tile([128, CH], F32, tag="in1e")
            sel5(in1[:], ps, se, B0 + NRZ, CH)
            hn1 = gp.tile([128, CH], F32, tag="hn1e")
            sel5(hn1[:], ps, se, C0, CH)
            acte = gp.tile([128, 128], F32, tag="acte")
            nc.scalar.activation(acte[:], rz1[:], AF.Sigmoid)
            rhne = gp.tile([128, CH], F32, tag="rhne")
            nc.gpsimd.tensor_mul(rhne[:], acte[:, 0:CH], hn1[:])
            nc.gpsimd.tensor_add(rhne[:], rhne[:], in1[:])
            nte = gp.tile([128, CH], F32, tag="nte")
            nc.scalar.activation(nte[:], rhne[:], AF.Tanh)
            tte = gp.tile([128, CH], F32, tag="tte")
            nc.gpsimd.tensor_tensor(tte[:], nte[:], hC[:, CH:128], op=ALU.subtract)
            ute = gp.tile([128, CH], F32, tag="ute")
            nc.gpsimd.tensor_mul(ute[:], acte[:, CH:128], tte[:])
            nc.gpsimd.tensor_tensor(tte[:], tte[:], ute[:], op=ALU.subtract)
            h1e = gp.tile([128, CH], F32, tag="h1e")
            nc.vector.scalar_tensor_tensor(
                h1e[:], tte[:], mT_sb[:, se:se + 1], hC[:, CH:128],
                op0=ALU.mult, op1=ALU.add)
            nc.sync.dma_start(d_outs[T - 1], h1e[:])
            nc.sync.dma_start(d_hfin[0], hC[:, 0:CH])
            nc.sync.dma_start(d_hfin[1], h1e[:])
    return nc


_CACHE = {}


def kernel(tokens, emb, W_ih0, W_hh0, b_ih0, b_hh0, W_ihL, W_hhL, b_ihL, b_hhL,
           _trace=False):
    tokens = np.asarray(tokens)
    emb = np.asarray(emb, np.float32)
    T = tokens.shape[0]

    shared, per_core = _prep(tokens, emb,
                             np.asarray(W_ih0, np.float32),
                             np.asarray(W_hh0, np.float32),
                             np.asarray(b_ih0, np.float32),
                             np.asarray(b_hh0, np.float32),
                             np.asarray(W_ihL, np.float32),
                             np.asarray(W_hhL, np.float32),
                             np.asarray(b_ihL, np.float32),
                             np.asarray(b_hhL, np.float32))

    if T not in _CACHE:
        nc = bacc.Bacc(None, num_devices=NCORES)
        build_program(nc, T)
        nc.finalize()
        _CACHE[T] = nc
    nc = _CACHE[T]

    in_maps = [{**shared, **per_core[c]} for c in range(NCORES)]
    res = run_bass_kernel_spmd(nc, in_maps, list(range(NCORES)), trace=_trace)

    outs = [res.results[c]["outs"] for c in range(NCORES)]   # [T,128,CH]
    hfin = [res.results[c]["hfin"] for c in range(NCORES)]   # [2,128,CH]

    outs_full = np.concatenate(outs, axis=2)                 # [T,128,512]
    outs_f = outs_full[:, 0:B, :H]                           # [T,B,H]
    outs_b = outs_full[:, B:128, :H]
    output = (outs_f[::-1] + outs_b) * 0.5

    hf = np.concatenate(hfin, axis=2)                        # [2,128,512]
    h0 = np.stack([hf[0, 0:B, :H], hf[1, 0:B, :H],
                   hf[0, B:128, :H], hf[1, B:128, :H]])      # [4,B,H]
    if _trace:
        return (output.astype(np.float32), h0.astype(np.float32)), res
    return output.astype(np.float32), h0.astype(np.float32)


# revision 13
# speedup vs baseline: 11.7492x; 1.0967x over previous
"""Trainium2 Bass kernel for nn_BidirRecurrentModel (moe_routing).

Bidirectional 2-layer GRU, T=256 steps, B=64, H=500, 6 experts routed by
token id (only experts 0..4 are reachable: e = tok // 10000 < 5).

Strategy (8 NeuronCores, SPMD):
  - Hidden dim H padded 500->512, sharded 8 ways (chunk of 64 h-dims per
    core).  Both directions stacked into the M=128 token dim (64 fwd +
    64 bwd), so every matmul runs with a full 128-wide stationary.
  - Each core holds the (r,z,n) weight row-slices for its chunk of all
    3 recurrent weight groups (W_hh0, W_ihL, W_hhL), SBUF-resident, in
    fp32r (full-rate fp32 matmul mode).
  - Layer-0 input gates gi0 depend only on token ids -> precomputed on
    device in a parallel phase (all-expert matmul + one-hot select)
    into a DRAM table, read back 2 rows per step.
  - Per step, one 8-core AllGather carries this step's h0_new chunk and
    the previous step's h1_new chunk (transposed), rebuilding the full
    h^T stationaries on every core.  Layer-1 biases ride inside the gi1
    matmul via one-hot rows appended to the stationary.
  - Expert selection: scalar_tensor_tensor accumulation with per-token
    one-hot columns; gi1+gh1 r,z pre-acts are fused by PSUM
    accumulation, n pre-acts kept separate (GRU needs r * hn).

kernel(**inputs) takes the FULL inputs (as produced by setup_inputs())
and returns (output [T,B,H], h0 [2L,B,H]) matching the reference.
"""

import numpy as np

import concourse.bacc as bacc
import concourse.mybir as mybir
import concourse.tile as tile
from concourse.bass_utils import run_bass_kernel_spmd

F32 = mybir.dt.float32
F32R = mybir.dt.float32r
AF = mybir.ActivationFunctionType
ALU = mybir.AluOpType

NCORES = 8
H = 500
HP = 512            # padded hidden
CH = HP // NCORES   # 64 h-dims per core
NE = 5              # reachable experts
B = 64
M = 2 * B           # fwd + bwd stacked
K = HP              # contraction (pad + bias/one-hot rows)
NRZ = NE * 2 * CH   # 640 r,z cols
NN = NE * CH        # 320 n cols
NW = NRZ + NN       # 960 streamed cols per group
EXPERT_DIV = 10000

# conservative-mode switches (validated fast paths get flipped on)
USE_VT = False      # DVE stream-transpose instead of PE transpose+copy
INPLACE_SEL = True  # in-place scalar_tensor_tensor accumulation


def _bf16_rne(a):
    u = a.view(np.uint32)
    r = ((u >> 16) + (((u >> 15) & 1) & ((u & 0x17FFF) != 0) * 1)).astype(np.uint32) << 16
    return r.view(np.float32)


def round_f32r(a):
    a = np.ascontiguousarray(a, np.float32)
    hi = _bf16_rne(a)
    lo = _bf16_rne((a - hi).astype(np.float32))
    return (hi + lo).astype(np.float32)


def _ktile(img):
    """[K, N] -> [128, K//128 * N] with k-tile k at cols [k*N, (k+1)*N)."""
    Kd, N = img.shape
    nk = Kd // 128
    return np.ascontiguousarray(
        img.reshape(nk, 128, N).transpose(1, 0, 2).reshape(128, nk * N)
    )


def _rz_col(e, j):
    return e * 2 * CH + j          # j in [0, 128): r then z


def _n_col(e, j):
    return NRZ + e * CH + j


def _build_weight_img(Wlist, bias_rz=None, bias_n=None, core=0):
    """Build [K, NW] streamed image for one weight group, rows for this
    core's h-chunk, cols [rz | n] expert-major.  Wlist: [E][1500, 500].
    bias_rz/bias_n: [E][1500] biases placed on one-hot row 500+e for the
    r,z / n column blocks respectively (the GRU applies hidden-side n
    bias inside r*(hn+b), so n biases must ride with their own matmul)."""
    img = np.zeros((K, NW), np.float32)
    c0 = core * CH
    for e in range(NE):
        W = Wlist[e]
        for bi, boff in ((0, 0), (1, 500), (2, 1000)):  # r, z, n
            rows = np.arange(c0, min(c0 + CH, 500))
            wsub = W[boff + rows, :500]                 # [nv, 500]
            if bi < 2:
                cols = _rz_col(e, bi * CH) + np.arange(len(rows))
                bias = bias_rz
            else:
                cols = _n_col(e, 0) + np.arange(len(rows))
                bias = bias_n
            img[:500, cols] = wsub.T
            if bias is not None:
                img[500 + e, cols] = bias[e][boff + rows]
    return round_f32r(img)


def _prep(tokens, emb, W_ih0, W_hh0, b_ih0, b_hh0, W_ihL, W_hhL, b_ihL, b_hhL):
    T = tokens.shape[0]
    tok = np.asarray(tokens)
    eid = (tok // EXPERT_DIV).astype(np.int64)        # [T, B]
    msk = (tok > 0).astype(np.float32)                # [T, B]

    # ---- per-core streamed weight images (k-tiled, fp32r-rounded) ----
    whh0_img, wihl_img, whhl_img, w0_img, bhn_tab = [], [], [], [], []
    biasL = [(b_ihL[e, 0] + b_hhL[e, 0]) for e in range(NE)]
    biasL_i = [b_ihL[e, 0] for e in range(NE)]
    biasL_h = [b_hhL[e, 0] for e in range(NE)]
    bias0 = [(b_ih0[e] + b_hh0[e]) for e in range(NE)]
    bias0_i = [b_ih0[e] for e in range(NE)]
    for c in range(NCORES):
        whh0_img.append(_ktile(_build_weight_img(
            [W_hh0[e] for e in range(NE)], None, None, c)))
        wihl_img.append(_ktile(_build_weight_img(
            [W_ihL[e, 0] for e in range(NE)], biasL, biasL_i, c)))
        whhl_img.append(_ktile(_build_weight_img(
            [W_hhL[e, 0] for e in range(NE)], None, biasL_h, c)))
        # layer-0 hidden-side n bias per token instance: [T*B, CH]
        c0 = c * CH
        nv = min(c0 + CH, 500) - c0
        bh = np.zeros((T * B, CH), np.float32)
        bh[:, :nv] = b_hh0[eid.reshape(-1)][:, 1000 + c0:1000 + c0 + nv]
        bhn_tab.append(bh)
        # parallel-phase image: W_ih0 with combined layer-0 bias on row 500
        img = np.zeros((K, NW), np.float32)
        for e in range(NE):
            for bi, boff in ((0, 0), (1, 500), (2, 1000)):
                rows = np.arange(c0, min(c0 + CH, 500))
                if bi < 2:
                    cols = _rz_col(e, bi * CH) + np.arange(len(rows))
                    bsrc = bias0[e]
                else:
                    cols = _n_col(e, 0) + np.arange(len(rows))
                    bsrc = bias0_i[e]
                img[:500, cols] = W_ih0[e][boff + rows, :500].T
                img[500, cols] = bsrc[boff + rows]
        w0_img.append(_ktile(round_f32r(img)))

    # ---- x_aug^T (shared): emb rows for all T*B instances ----
    x = np.asarray(emb)[tok.reshape(-1)].astype(np.float32)   # [T*B, 500]
    x_aug = np.zeros((T * B, K), np.float32)
    x_aug[:, :500] = x
    x_aug[:, 500] = 1.0
    x_augT = _ktile(round_f32r(np.ascontiguousarray(x_aug.T)))  # [128, 4*T*B]

    # ---- one-hot / mask tables ----
    # parallel selects: [128, (T*B/128) * 5]
    ntile = T * B // 128
    einst = eid.reshape(-1)
    ohp = np.zeros((128, ntile * NE), np.float32)
    for j in range(ntile):
        es = einst[j * 128:(j + 1) * 128]
        ohp[np.arange(128), j * NE + es] = 1.0
    # sequential selects: [128, T*5], rows 0:64 fwd step s, 64:128 bwd
    ohs = np.zeros((128, T * NE), np.float32)
    for s in range(T):
        ohs[np.arange(B), s * NE + eid[s]] = 1.0
        ohs[B + np.arange(B), s * NE + eid[T - 1 - s]] = 1.0
    # one-hot rows DMA'd into X0 k-tile 3 partitions 116..120: [T, 5, 128]
    ohT = np.zeros((T, NE, 128), np.float32)
    for s in range(T):
        ohT[s, eid[s], np.arange(B)] = 1.0
        ohT[s, eid[T - 1 - s], B + np.arange(B)] = 1.0
    # masks [128, T]
    mT = np.zeros((128, T), np.float32)
    for s in range(T):
        mT[:B, s] = msk[s]
        mT[B:, s] = msk[T - 1 - s]

    shared = {
        "x_augT": x_augT, "ohp": ohp, "ohs": ohs, "ohT": ohT, "mT": mT,
        "ident": np.eye(128, dtype=np.float32),
    }
    per_core = [
        {"whh0_img": whh0_img[c], "wihl_img": wihl_img[c],
         "whhl_img": whhl_img[c], "w0_img": w0_img[c], "bhn": bhn_tab[c]}
        for c in range(NCORES)
    ]
    return shared, per_core


def build_program(nc, T):
    nk = K // 128
    NI = T * B          # token instances
    ntile = NI // 128

    # ---- I/O ----
    d_whh0 = nc.dram_tensor("whh0_img", [128, nk * NW], F32R, kind="ExternalInput")
    d_wihl = nc.dram_tensor("wihl_img", [128, nk * NW], F32R, kind="ExternalInput")
    d_whhl = nc.dram_tensor("whhl_img", [128, nk * NW], F32R, kind="ExternalInput")
    d_w0 = nc.dram_tensor("w0_img", [128, nk * NW], F32R, kind="ExternalInput")
    d_xT = nc.dram_tensor("x_augT", [128, nk * NI], F32R, kind="ExternalInput")
    d_ohp = nc.dram_tensor("ohp", [128, ntile * NE], F32, kind="ExternalInput")
    d_ohs = nc.dram_tensor("ohs", [128, T * NE], F32, kind="ExternalInput")
    d_ohT = nc.dram_tensor("ohT", [T, NE, 128], F32R, kind="ExternalInput")
    d_mT = nc.dram_tensor("mT", [128, T], F32, kind="ExternalInput")
    d_ident = nc.dram_tensor("ident", [128, 128], F32, kind="ExternalInput")
    d_bhn = nc.dram_tensor("bhn", [T * B, CH], F32, kind="ExternalInput")
    d_outs = nc.dram_tensor("outs", [T, 128, CH], F32, kind="ExternalOutput")
    d_hfin = nc.dram_tensor("hfin", [2, 128, CH], F32, kind="ExternalOutput")

    gi0_tab = nc.dram_tensor("gi0_tab", [NI, 3 * CH], F32)   # internal

    with tile.TileContext(nc) as tc:
        with (
            tc.tile_pool(name="const", bufs=1) as cp,
            tc.tile_pool(name="seq", bufs=3) as sq,
            tc.tile_pool(name="gates", bufs=3) as gp,
            tc.tile_pool(name="par", bufs=3) as pp,
            tc.tile_pool(name="psA", bufs=1, space="PSUM") as psA,
            tc.tile_pool(name="psB", bufs=1, space="PSUM") as psB,
            tc.tile_pool(name="psC", bufs=1, space="PSUM") as psC,
            tc.tile_pool(name="dram", bufs=3, space="DRAM") as dr,
        ):
            # ---------- persistent SBUF ----------
            w_hh0 = cp.tile([128, nk * NW], F32R, tag="w_hh0")
            w_ihl = cp.tile([128, nk * NW], F32R, tag="w_ihl")
            w_hhl = cp.tile([128, nk * NW], F32R, tag="w_hhl")
            w_0 = cp.tile([128, nk * NW], F32R, tag="w_0")
            ohs_sb = cp.tile([128, T * NE], F32, tag="ohs")
            ohp_sb = cp.tile([128, ntile * NE], F32, tag="ohp")
            mT_sb = cp.tile([128, T], F32, tag="mT")
            X0 = cp.tile([128, nk * M], F32R, tag="X0")
            X1 = cp.tile([128, nk * M], F32R, tag="X1")
            zer = cp.tile([B, M], F32, tag="zer")

            nc.sync.dma_start(w_hh0[:], d_whh0[:])
            nc.sync.dma_start(w_ihl[:], d_wihl[:])
            nc.sync.dma_start(w_hhl[:], d_whhl[:])
            nc.sync.dma_start(w_0[:], d_w0[:])
            nc.sync.dma_start(ohs_sb[:], d_ohs[:])
            nc.sync.dma_start(ohp_sb[:], d_ohp[:])
            nc.sync.dma_start(mT_sb[:], d_mT[:])
            nc.vector.memset(X0[:].bitcast(F32), 0.0)
            nc.vector.memset(X1[:].bitcast(F32), 0.0)
            nc.vector.memset(zer[:], 0.0)

            ident_sb = cp.tile([128, 128], F32, tag="ident")
            nc.sync.dma_start(ident_sb[:], d_ident[:])

            # ---------- parallel phase: gi0 table ----------
            for j in range(ntile):
                xs = pp.tile([128, nk * 128], F32R, tag="xs")
                nc.sync.dma_start(
                    xs[:].rearrange("p (k m) -> p k m", k=nk),
                    d_xT[:].rearrange("p (k i) -> p k i", k=nk)[:, :, j * 128:(j + 1) * 128])
                pm = psA.tile([128, 1024], F32, tag="pmA")
                for n0, n1 in ((0, 512), (512, NW)):
                    for k in range(nk):
                        nc.tensor.matmul(
                            pm[:, n0:n1],
                            xs[:, k * 128:(k + 1) * 128],
                            w_0[:, k * NW + n0:k * NW + n1],
                            start=(k == 0), stop=(k == nk - 1))
                gsel = pp.tile([128, 3 * CH], F32, tag="gsel")
                # rz select (5 experts) then n select
                for e in range(NE):
                    oh = ohp_sb[:, j * NE + e:j * NE + e + 1]
                    if e == 0:
                        nc.vector.tensor_scalar_mul(gsel[:, 0:128], pm[:, 0:128], oh)
                        nc.vector.tensor_scalar_mul(
                            gsel[:, 128:192], pm[:, NRZ:NRZ + CH], oh)
                    else:
                        nc.vector.scalar_tensor_tensor(
                            gsel[:, 0:128], pm[:, e * 128:(e + 1) * 128], oh,
                            gsel[:, 0:128], op0=ALU.mult, op1=ALU.add)
                        nc.vector.scalar_tensor_tensor(
                            gsel[:, 128:192], pm[:, NRZ + e * CH:NRZ + (e + 1) * CH],
                            oh, gsel[:, 128:192], op0=ALU.mult, op1=ALU.add)
                nc.sync.dma_start(
                    gi0_tab[j * 128:(j + 1) * 128, :], gsel[:])

            # ---------- helpers ----------
            # PSUM bank layout (one [128, 3072] tile = 6 banks):
            #   A (gh0)          cols    0: 960  [rz 640 | n 320]
            #   B (gi1 + gh1rz)  cols 1024:1984  [rz 640 | n 320]
            #   C (gh1 n)        cols 2048:2368
            #   Tr (transpose)   cols 2560:2688
            A0, B0, C0, T0 = 0, 1024, 2048, 2560

            def mm_seq(ps, X0t, X1t):
                # PE order: C first (frees hn1 select early), then A, then B
                for n0, n1, img, stat, acc in (
                        (C0, C0 + 320, w_hhl, X1t, "n"),      # gh1 n
                        (A0, A0 + 512, w_hh0, X0t, None),
                        (A0 + 512, A0 + 960, w_hh0, X0t, None),
                        (B0, B0 + 512, w_ihl, X0t, None),
                        (B0 + 512, B0 + 960, w_ihl, X0t, None),
                        (B0, B0 + 512, w_hhl, X1t, "acc"),     # gh1 rz accum
                        (B0 + 512, B0 + 640, w_hhl, X1t, "acc")):
                    base = {None: A0 if n0 < 1024 else B0, "acc": B0, "n": C0 - NRZ}[acc]
                    woff = (n0 - base) if acc != "n" else NRZ + (n0 - C0)
                    for k in range(nk):
                        nc.tensor.matmul(
                            ps[:, n0:n1],
                            stat[:, k * M:(k + 1) * M],
                            img[:, k * NW + woff:k * NW + woff + (n1 - n0)],
                            start=(k == 0 and acc != "acc"), stop=(k == nk - 1))

            def sel5(dst, ps, scol, col0, width, base=None):
                for e in range(NE):
                    oh = ohs_sb[:, scol * NE + e:scol * NE + e + 1]
                    src = ps[:, col0 + e * width:col0 + (e + 1) * width]
                    if e == 0 and base is None:
                        nc.vector.tensor_scalar_mul(dst, src, oh)
                    else:
                        nc.vector.scalar_tensor_tensor(
                            dst, src, oh, dst if e else base,
                            op0=ALU.mult, op1=ALU.add)

            def readback(ago):
                # agout rows = (rank, layer, ch); h-dim d = rank*CH + ch.
                # X k-tile k partition p <- d = k*128+p: ranks 2k, 2k+1.
                # One DMA per (X, k): 2 contiguous 32KB blocks each.
                for k in range(nk):
                    for r2 in range(2):
                        row = (2 * k + r2) * 128
                        nc.gpsimd.dma_start(
                            X1[r2 * CH:(r2 + 1) * CH, k * M:(k + 1) * M],
                            ago[row + CH:row + 2 * CH, :])
                for k in range(nk):
                    for r2 in range(2):
                        row = (2 * k + r2) * 128
                        nc.sync.dma_start(
                            X0[r2 * CH:(r2 + 1) * CH, k * M:(k + 1) * M],
                            ago[row:row + CH, :])

            # ---------- sequential phase ----------
            agout_tiles = {}
            hC = gp.tile([128, 128], F32, tag="hC")   # [h0 | h1] local
            nc.vector.memset(hC[:], 0.0)

            for s in range(T):
                if s > 0:
                    ago = agout_tiles.pop(s - 1)
                    readback(ago)
                    nc.sync.dma_start(X0[116:121, 3 * M:4 * M], d_ohT[s - 1])
                    nc.sync.dma_start(X1[116:121, 3 * M:4 * M], d_ohT[s - 1])
                sc1 = max(s - 1, 0)   # L1 lane step index (dummy at s=0)

                ps = psA.tile([128, 3072], F32, tag="ps")
                mm_seq(ps, X0, X1)

                gi0 = sq.tile([128, 128], F32, tag="gi0")
                nc.sync.dma_start(gi0[0:B, :], gi0_tab[s * B:(s + 1) * B, 0:128])
                nc.sync.dma_start(
                    gi0[B:128, :], gi0_tab[(T - 1 - s) * B:(T - s) * B, 0:128])
                # tabs: [inn0 | bhn] (DMA-only tile, no compute writers)
                tabs = sq.tile([128, 2 * CH], F32, tag="tabs")
                nc.sync.dma_start(
                    tabs[0:B, 0:CH], gi0_tab[s * B:(s + 1) * B, 128:192])
                nc.sync.dma_start(
                    tabs[B:128, 0:CH],
                    gi0_tab[(T - 1 - s) * B:(T - s) * B, 128:192])
                nc.sync.dma_start(
                    tabs[0:B, CH:2 * CH], d_bhn[s * B:(s + 1) * B, :])
                nc.sync.dma_start(
                    tabs[B:128, CH:2 * CH],
                    d_bhn[(T - 1 - s) * B:(T - s) * B, :])

                # selects -> rzC [L0 | L1], nCs [inn1 | hn0 | hn1]
                rzC = gp.tile([128, 256], F32, tag="rzC")
                nCs = gp.tile([128, 192], F32, tag="nCs")
                sel5(nCs[:, 128:192], ps, sc1, C0, CH)                 # hn1
                sel5(rzC[:, 0:128], ps, s, A0, 128, base=gi0[:])       # rz0
                sel5(nCs[:, 64:128], ps, s, A0 + NRZ, CH,
                     base=tabs[:, CH:2 * CH])                          # hn0
                sel5(rzC[:, 128:256], ps, sc1, B0, 128)                # rz1
                sel5(nCs[:, 0:CH], ps, sc1, B0 + NRZ, CH)              # inn1

                # combined gates ([128,128] ops, lanes L0|L1)
                act = gp.tile([128, 256], F32, tag="act")
                nc.scalar.activation(act[:], rzC[:], AF.Sigmoid)
                lrz = act[:].rearrange("p (l rz c) -> p l rz c", l=2, rz=2)
                r_ap, z_ap = lrz[:, :, 0, :], lrz[:, :, 1, :]
                rhn = gp.tile([128, 128], F32, tag="rhn")
                rhn2 = rhn[:].rearrange("p (l c) -> p l c", l=2)
                nc.gpsimd.tensor_tensor(
                    rhn2, r_ap,
                    nCs[:, 64:192].rearrange("p (l c) -> p l c", l=2),
                    op=ALU.mult)
                nc.gpsimd.tensor_tensor(rhn[:, 0:CH], rhn[:, 0:CH],
                                        tabs[:, 0:CH], op=ALU.add)
                nc.gpsimd.tensor_tensor(rhn[:, CH:128], rhn[:, CH:128],
                                        nCs[:, 0:CH], op=ALU.add)
                nt = gp.tile([128, 128], F32, tag="nt")
                nc.scalar.activation(nt[:], rhn[:], AF.Tanh)
                tt_ = gp.tile([128, 128], F32, tag="tt_")
                nc.gpsimd.tensor_tensor(tt_[:], nt[:], hC[:], op=ALU.subtract)
                ut = gp.tile([128, 128], F32, tag="ut")
                nc.gpsimd.tensor_tensor(
                    ut[:].rearrange("p (l c) -> p l c", l=2), z_ap,
                    tt_[:].rearrange("p (l c) -> p l c", l=2), op=ALU.mult)
                nc.gpsimd.tensor_tensor(tt_[:], tt_[:], ut[:],
                                        op=ALU.subtract)
                hN = gp.tile([128, 128], F32, tag="hC")
                nc.vector.scalar_tensor_tensor(
                    hN[:, 0:CH], tt_[:, 0:CH], mT_sb[:, s:s + 1],
                    hC[:, 0:CH], op0=ALU.mult, op1=ALU.add)
                nc.vector.scalar_tensor_tensor(
                    hN[:, CH:128], tt_[:, CH:128], mT_sb[:, sc1:sc1 + 1],
                    hC[:, CH:128], op0=ALU.mult, op1=ALU.add)
                if s > 0:
                    nc.gpsimd.dma_start(d_outs[s - 1], hN[:, CH:128])
                hC = hN

                # transpose both halves at once -> agin
                nc.tensor.transpose(ps[:, T0:T0 + 128], hN[:], ident_sb[:])
                ttile = gp.tile([128, 128], F32R, tag="ttile")
                nc.scalar.copy(ttile[:], ps[:, T0:T0 + 128])
                agin = dr.tile([M, 128], F32R, tag="agin")
                nc.sync.dma_start(agin[:], ttile[:])

                agout = dr.tile([NCORES * M, 128], F32R, tag="agout")
                agout_tiles[s] = agout
                nc.gpsimd.collective_compute(
                    "AllGather", ALU.bypass,
                    replica_groups=[list(range(NCORES))],
                    ins=[agin.opt()], outs=[agout.opt()])

            # ---------- epilogue: L1(T-1) ----------
            ago = agout_tiles.pop(T - 1)
            readback(ago)
            nc.sync.dma_start(X0[116:121, 3 * M:4 * M], d_ohT[T - 1])
            nc.sync.dma_start(X1[116:121, 3 * M:4 * M], d_ohT[T - 1])
            ps = psA.tile([128, 3072], F32, tag="ps")
            mm_seq(ps, X0, X1)
            se = T - 1
            rz1 = gp.tile([128, 128], F32, tag="rzC")
            sel5(rz1[:], ps, se, B0, 128)
            in1 = gp.tile([128, CH], F32, tag="in1e")
            sel5(in1[:], ps, se, B0 + NRZ, CH)
            hn1 = gp.tile([128, CH], F32, tag="hn1e")
            sel5(hn1[:], ps, se, C0, CH)
            acte = gp.tile([128, 128], F32, tag="acte")
            nc.scalar.activation(acte[:], rz1[:], AF.Sigmoid)
            rhne = gp.tile([128, CH], F32, tag="rhne")
            nc.gpsimd.tensor_mul(rhne[:], acte[:, 0:CH], hn1[:])
            nc.gpsimd.tensor_add(rhne[:], rhne[:], in1[:])
            nte = gp.tile([128, CH], F32, tag="nte")
            nc.scalar.activation(nte[:], rhne[:], AF.Tanh)
            tte = gp.tile([128, CH], F32, tag="tte")
            nc.gpsimd.tensor_tensor(tte[:], nte[:], hC[:, CH:128], op=ALU.subtract)
            ute = gp.tile([128, CH], F32, tag="ute")
            nc.gpsimd.tensor_mul(ute[:], acte[:, CH:128], tte[:])
            nc.gpsimd.tensor_tensor(tte[:], tte[:], ute[:], op=ALU.subtract)
            h1e = gp.tile([128, CH], F32, tag="h1e")
            nc.vector.scalar_tensor_tensor(
                h1e[:], tte[:], mT_sb[:, se:se + 1], hC[:, CH:128],
                op0=ALU.mult, op1=ALU.add)
            nc.sync.dma_start(d_outs[T - 1], h1e[:])
            nc.sync.dma_start(d_hfin[0], hC[:, 0:CH])
            nc.sync.dma_start(d_hfin[1], h1e[:])
    return nc


_CACHE = {}


def kernel(tokens, emb, W_ih0, W_hh0, b_ih0, b_hh0, W_ihL, W_hhL, b_ihL, b_hhL,
           _trace=False):
    tokens = np.asarray(tokens)
    emb = np.asarray(emb, np.float32)
    T = tokens.shape[0]

    shared, per_core = _prep(tokens, emb,
                             np.asarray(W_ih0, np.float32),
                             np.asarray(W_hh0, np.float32),
                             np.asarray(b_ih0, np.float32),
                             np.asarray(b_hh0, np.float32),
                             np.asarray(W_ihL, np.float32),
                             np.asarray(W_hhL, np.float32),
                             np.asarray(b_ihL, np.float32),
                             np.asarray(b_hhL, np.float32))

    if T not in _CACHE:
        nc = bacc.Bacc(None, num_devices=NCORES)
        build_program(nc, T)
        nc.finalize()
        _CACHE[T] = nc
    nc = _CACHE[T]

    in_maps = [{**shared, **per_core[c]} for c in range(NCORES)]
    res = run_bass_kernel_spmd(nc, in_maps, list(range(NCORES)), trace=_trace)

    outs = [res.results[c]["outs"] for c in range(NCORES)]   # [T,128,CH]
    hfin = [res.results[c]["hfin"] for c in range(NCORES)]   # [2,128,CH]

    outs_full = np.concatenate(outs, axis=2)                 # [T,128,512]
    outs_f = outs_full[:, 0:B, :H]                           # [T,B,H]
    outs_b = outs_full[:, B:128, :H]
    output = (outs_f[::-1] + outs_b) * 0.5

    hf = np.concatenate(hfin, axis=2)                        # [2,128,512]
    h0 = np.stack([hf[0, 0:B, :H], hf[1, 0:B, :H],
                   hf[0, B:128, :H], hf[1, B:128, :H]])      # [4,B,H]
    if _trace:
        return (output.astype(np.float32), h0.astype(np.float32)), res
    return output.astype(np.float32), h0.astype(np.float32)


# revision 16
# speedup vs baseline: 14.1078x; 1.2007x over previous
"""Trainium2 Bass kernel for nn_BidirRecurrentModel (moe_routing).

Bidirectional 2-layer GRU, T=256 steps, B=64, H=500, 6 experts routed by
token id (only experts 0..4 are reachable: e = tok // 10000 < 5).

Strategy (8 NeuronCores, SPMD):
  - Hidden dim H padded 500->512, sharded 8 ways (chunk of 64 h-dims per
    core).  Both directions stacked into the M=128 token dim (64 fwd +
    64 bwd), so every matmul runs with a full 128-wide stationary.
  - Each core holds the (r,z,n) weight row-slices for its chunk of all
    3 recurrent weight groups (W_hh0, W_ihL, W_hhL), SBUF-resident, in
    fp32r (full-rate fp32 matmul mode).
  - Layer-0 input gates gi0 depend only on token ids -> precomputed on
    device in a parallel phase (all-expert matmul + one-hot select)
    into a DRAM table, read back 2 rows per step.
  - Per step, one 8-core AllGather carries this step's h0_new chunk and
    the previous step's h1_new chunk (transposed), rebuilding the full
    h^T stationaries on every core.  Layer-1 biases ride inside the gi1
    matmul via one-hot rows appended to the stationary.
  - Expert selection: scalar_tensor_tensor accumulation with per-token
    one-hot columns; gi1+gh1 r,z pre-acts are fused by PSUM
    accumulation, n pre-acts kept separate (GRU needs r * hn).

kernel(**inputs) takes the FULL inputs (as produced by setup_inputs())
and returns (output [T,B,H], h0 [2L,B,H]) matching the reference.
"""

import numpy as np

import concourse.bacc as bacc
import concourse.mybir as mybir
import concourse.tile as tile
from concourse.bass_utils import run_bass_kernel_spmd

F32 = mybir.dt.float32
F32R = mybir.dt.float32r
AF = mybir.ActivationFunctionType
ALU = mybir.AluOpType

NCORES = 8
H = 500
HP = 512            # padded hidden
CH = HP // NCORES   # 64 h-dims per core
NE = 5              # reachable experts
B = 64
M = 2 * B           # fwd + bwd stacked
K = HP              # contraction (pad + bias/one-hot rows)
NRZ = NE * 2 * CH   # 640 r,z cols
NN = NE * CH        # 320 n cols
NW = NRZ + NN       # 960 streamed cols per group
EXPERT_DIV = 10000

# conservative-mode switches (validated fast paths get flipped on)
USE_VT = False      # DVE stream-transpose instead of PE transpose+copy
INPLACE_SEL = True  # in-place scalar_tensor_tensor accumulation


def _bf16_rne(a):
    u = a.view(np.uint32)
    r = ((u >> 16) + (((u >> 15) & 1) & ((u & 0x17FFF) != 0) * 1)).astype(np.uint32) << 16
    return r.view(np.float32)


def round_f32r(a):
    a = np.ascontiguousarray(a, np.float32)
    hi = _bf16_rne(a)
    lo = _bf16_rne((a - hi).astype(np.float32))
    return (hi + lo).astype(np.float32)


def _ktile(img):
    """[K, N] -> [128, K//128 * N] with k-tile k at cols [k*N, (k+1)*N)."""
    Kd, N = img.shape
    nk = Kd // 128
    return np.ascontiguousarray(
        img.reshape(nk, 128, N).transpose(1, 0, 2).reshape(128, nk * N)
    )


def _rz_col(e, j):
    return e * 2 * CH + j          # j in [0, 128): r then z


def _n_col(e, j):
    return NRZ + e * CH + j


def _build_weight_img(Wlist, bias_rz=None, bias_n=None, core=0):
    """Build [K, NW] streamed image for one weight group, rows for this
    core's h-chunk, cols [rz | n] expert-major.  Wlist: [E][1500, 500].
    bias_rz/bias_n: [E][1500] biases placed on one-hot row 500+e for the
    r,z / n column blocks respectively (the GRU applies hidden-side n
    bias inside r*(hn+b), so n biases must ride with their own matmul)."""
    img = np.zeros((K, NW), np.float32)
    c0 = core * CH
    for e in range(NE):
        W = Wlist[e]
        for bi, boff in ((0, 0), (1, 500), (2, 1000)):  # r, z, n
            rows = np.arange(c0, min(c0 + CH, 500))
            wsub = W[boff + rows, :500]                 # [nv, 500]
            if bi < 2:
                cols = _rz_col(e, bi * CH) + np.arange(len(rows))
                bias = bias_rz
            else:
                cols = _n_col(e, 0) + np.arange(len(rows))
                bias = bias_n
            img[:500, cols] = wsub.T
            if bias is not None:
                img[500 + e, cols] = bias[e][boff + rows]
    return round_f32r(img)


def _prep(tokens, emb, W_ih0, W_hh0, b_ih0, b_hh0, W_ihL, W_hhL, b_ihL, b_hhL):
    T = tokens.shape[0]
    tok = np.asarray(tokens)
    eid = (tok // EXPERT_DIV).astype(np.int64)        # [T, B]
    msk = (tok > 0).astype(np.float32)                # [T, B]

    # ---- per-core streamed weight images (k-tiled, fp32r-rounded) ----
    whh0_img, wihl_img, whhl_img, w0_img, bhn_tab = [], [], [], [], []
    biasL = [(b_ihL[e, 0] + b_hhL[e, 0]) for e in range(NE)]
    biasL_i = [b_ihL[e, 0] for e in range(NE)]
    biasL_h = [b_hhL[e, 0] for e in range(NE)]
    bias0 = [(b_ih0[e] + b_hh0[e]) for e in range(NE)]
    bias0_i = [b_ih0[e] for e in range(NE)]
    for c in range(NCORES):
        whh0_img.append(_ktile(_build_weight_img(
            [W_hh0[e] for e in range(NE)], None, None, c)))
        wihl_img.append(_ktile(_build_weight_img(
            [W_ihL[e, 0] for e in range(NE)], biasL, biasL_i, c)))
        whhl_img.append(_ktile(_build_weight_img(
            [W_hhL[e, 0] for e in range(NE)], None, biasL_h, c)))
        # layer-0 hidden-side n bias per token instance: [T*B, CH]
        c0 = c * CH
        nv = min(c0 + CH, 500) - c0
        bh = np.zeros((T * B, CH), np.float32)
        bh[:, :nv] = b_hh0[eid.reshape(-1)][:, 1000 + c0:1000 + c0 + nv]
        bhn_tab.append(bh)
        # parallel-phase image: W_ih0 with combined layer-0 bias on row 500
        img = np.zeros((K, NW), np.float32)
        for e in range(NE):
            for bi, boff in ((0, 0), (1, 500), (2, 1000)):
                rows = np.arange(c0, min(c0 + CH, 500))
                if bi < 2:
                    cols = _rz_col(e, bi * CH) + np.arange(len(rows))
                    bsrc = bias0[e]
                else:
                    cols = _n_col(e, 0) + np.arange(len(rows))
                    bsrc = bias0_i[e]
                img[:500, cols] = W_ih0[e][boff + rows, :500].T
                img[500, cols] = bsrc[boff + rows]
        w0_img.append(_ktile(round_f32r(img)))

    # ---- x_aug^T (shared): emb rows for all T*B instances ----
    x = np.asarray(emb)[tok.reshape(-1)].astype(np.float32)   # [T*B, 500]
    x_aug = np.zeros((T * B, K), np.float32)
    x_aug[:, :500] = x
    x_aug[:, 500] = 1.0
    x_augT = _ktile(round_f32r(np.ascontiguousarray(x_aug.T)))  # [128, 4*T*B]

    # ---- one-hot / mask tables ----
    # parallel selects: [128, (T*B/128) * 5]
    ntile = T * B // 128
    einst = eid.reshape(-1)
    ohp = np.zeros((128, ntile * NE), np.float32)
    for j in range(ntile):
        es = einst[j * 128:(j + 1) * 128]
        ohp[np.arange(128), j * NE + es] = 1.0
    # sequential selects: [128, T*5], rows 0:64 fwd step s, 64:128 bwd
    ohs = np.zeros((128, T * NE), np.float32)
    for s in range(T):
        ohs[np.arange(B), s * NE + eid[s]] = 1.0
        ohs[B + np.arange(B), s * NE + eid[T - 1 - s]] = 1.0
    # one-hot rows DMA'd into X0 k-tile 3 partitions 116..120: [T, 5, 128]
    ohT = np.zeros((T, NE, 128), np.float32)
    for s in range(T):
        ohT[s, eid[s], np.arange(B)] = 1.0
        ohT[s, eid[T - 1 - s], B + np.arange(B)] = 1.0
    # masks [128, T]
    mT = np.zeros((128, T), np.float32)
    for s in range(T):
        mT[:B, s] = msk[s]
        mT[B:, s] = msk[T - 1 - s]

    shared = {
        "x_augT": x_augT, "ohp": ohp, "ohs": ohs, "ohT": ohT, "mT": mT,
        "ident": np.eye(128, dtype=np.float32),
    }
    per_core = [
        {"whh0_img": whh0_img[c], "wihl_img": wihl_img[c],
         "whhl_img": whhl_img[c], "w0_img": w0_img[c], "bhn": bhn_tab[c]}
        for c in range(NCORES)
    ]
    return shared, per_core


def build_program(nc, T):
    nk = K // 128
    NI = T * B          # token instances
    ntile = NI // 128

    # ---- I/O ----
    d_whh0 = nc.dram_tensor("whh0_img", [128, nk * NW], F32R, kind="ExternalInput")
    d_wihl = nc.dram_tensor("wihl_img", [128, nk * NW], F32R, kind="ExternalInput")
    d_whhl = nc.dram_tensor("whhl_img", [128, nk * NW], F32R, kind="ExternalInput")
    d_w0 = nc.dram_tensor("w0_img", [128, nk * NW], F32R, kind="ExternalInput")
    d_xT = nc.dram_tensor("x_augT", [128, nk * NI], F32R, kind="ExternalInput")
    d_ohp = nc.dram_tensor("ohp", [128, ntile * NE], F32, kind="ExternalInput")
    d_ohs = nc.dram_tensor("ohs", [128, T * NE], F32, kind="ExternalInput")
    d_ohT = nc.dram_tensor("ohT", [T, NE, 128], F32R, kind="ExternalInput")
    d_mT = nc.dram_tensor("mT", [128, T], F32, kind="ExternalInput")
    d_ident = nc.dram_tensor("ident", [128, 128], F32, kind="ExternalInput")
    d_bhn = nc.dram_tensor("bhn", [T * B, CH], F32, kind="ExternalInput")
    d_outs = nc.dram_tensor("outs", [T, 128, CH], F32, kind="ExternalOutput")
    d_hfin = nc.dram_tensor("hfin", [2, 128, CH], F32, kind="ExternalOutput")

    gi0_tab = nc.dram_tensor("gi0_tab", [NI, 4 * CH], F32)   # internal

    with tile.TileContext(nc) as tc:
        with (
            tc.tile_pool(name="const", bufs=1) as cp,
            tc.tile_pool(name="seq", bufs=3) as sq,
            tc.tile_pool(name="gates", bufs=3) as gp,
            tc.tile_pool(name="par", bufs=3) as pp,
            tc.tile_pool(name="psA", bufs=1, space="PSUM") as psA,
            tc.tile_pool(name="psB", bufs=1, space="PSUM") as psB,
            tc.tile_pool(name="psC", bufs=1, space="PSUM") as psC,
            tc.tile_pool(name="dram", bufs=3, space="DRAM") as dr,
        ):
            # ---------- persistent SBUF ----------
            w_hh0 = cp.tile([128, nk * NW], F32R, tag="w_hh0")
            w_ihl = cp.tile([128, nk * NW], F32R, tag="w_ihl")
            w_hhl = cp.tile([128, nk * NW], F32R, tag="w_hhl")
            w_0 = cp.tile([128, nk * NW], F32R, tag="w_0")
            ohs_sb = cp.tile([128, T * NE], F32, tag="ohs")
            ohp_sb = cp.tile([128, ntile * NE], F32, tag="ohp")
            mT_sb = cp.tile([128, T], F32, tag="mT")
            X0 = cp.tile([128, nk * M], F32R, tag="X0")
            X1 = cp.tile([128, nk * M], F32R, tag="X1")
            zer = cp.tile([B, M], F32, tag="zer")

            nc.sync.dma_start(w_hh0[:], d_whh0[:])
            nc.sync.dma_start(w_ihl[:], d_wihl[:])
            nc.sync.dma_start(w_hhl[:], d_whhl[:])
            nc.sync.dma_start(w_0[:], d_w0[:])
            nc.sync.dma_start(ohs_sb[:], d_ohs[:])
            nc.sync.dma_start(ohp_sb[:], d_ohp[:])
            nc.sync.dma_start(mT_sb[:], d_mT[:])
            nc.vector.memset(X0[:].bitcast(F32), 0.0)
            nc.vector.memset(X1[:].bitcast(F32), 0.0)
            nc.vector.memset(zer[:], 0.0)

            ident_sb = cp.tile([128, 128], F32, tag="ident")
            nc.sync.dma_start(ident_sb[:], d_ident[:])

            # ---------- parallel phase: gi0 table ----------
            nc.sync.dma_start(gi0_tab[:, 192:256], d_bhn[:])
            for j in range(ntile):
                xs = pp.tile([128, nk * 128], F32R, tag="xs")
                nc.sync.dma_start(
                    xs[:].rearrange("p (k m) -> p k m", k=nk),
                    d_xT[:].rearrange("p (k i) -> p k i", k=nk)[:, :, j * 128:(j + 1) * 128])
                pm = psA.tile([128, 1024], F32, tag="pmA")
                for n0, n1 in ((0, 512), (512, NW)):
                    for k in range(nk):
                        nc.tensor.matmul(
                            pm[:, n0:n1],
                            xs[:, k * 128:(k + 1) * 128],
                            w_0[:, k * NW + n0:k * NW + n1],
                            start=(k == 0), stop=(k == nk - 1))
                gsel = pp.tile([128, 3 * CH], F32, tag="gsel")
                # rz select (5 experts) then n select
                for e in range(NE):
                    oh = ohp_sb[:, j * NE + e:j * NE + e + 1]
                    if e == 0:
                        nc.vector.tensor_scalar_mul(gsel[:, 0:128], pm[:, 0:128], oh)
                        nc.vector.tensor_scalar_mul(
                            gsel[:, 128:192], pm[:, NRZ:NRZ + CH], oh)
                    else:
                        nc.vector.scalar_tensor_tensor(
                            gsel[:, 0:128], pm[:, e * 128:(e + 1) * 128], oh,
                            gsel[:, 0:128], op0=ALU.mult, op1=ALU.add)
                        nc.vector.scalar_tensor_tensor(
                            gsel[:, 128:192], pm[:, NRZ + e * CH:NRZ + (e + 1) * CH],
                            oh, gsel[:, 128:192], op0=ALU.mult, op1=ALU.add)
                nc.sync.dma_start(
                    gi0_tab[j * 128:(j + 1) * 128, 0:192], gsel[:])

            # ---------- helpers ----------
            # Separate PSUM tiles so Tile tracks deps per group:
            #   A (gh0) [128,1024]=2 banks, B (gi1+gh1rz) 2 banks,
            #   C (gh1 n) 1 bank, Tr (transpose) 1 bank.
            def mm_seq(pA, pB, pC, X0t, X1t):
                # PE order: C first, then A, then B, then B-accum
                for dst, n0, n1, img, woff, stat, acc in (
                        (pC, 0, 320, w_hhl, NRZ, X1t, False),
                        (pA, 0, 512, w_hh0, 0, X0t, False),
                        (pA, 512, 960, w_hh0, 512, X0t, False),
                        (pB, 0, 512, w_ihl, 0, X0t, False),
                        (pB, 512, 960, w_ihl, 512, X0t, False),
                        (pB, 0, 512, w_hhl, 0, X1t, True),
                        (pB, 512, 640, w_hhl, 512, X1t, True)):
                    for k in range(nk):
                        nc.tensor.matmul(
                            dst[:, n0:n1],
                            stat[:, k * M:(k + 1) * M],
                            img[:, k * NW + woff:k * NW + woff + (n1 - n0)],
                            start=(k == 0 and not acc), stop=(k == nk - 1))

            def sel5(dst, ps, scol, col0, width, base=None):
                for e in range(NE):
                    oh = ohs_sb[:, scol * NE + e:scol * NE + e + 1]
                    src = ps[:, col0 + e * width:col0 + (e + 1) * width]
                    if e == 0 and base is None:
                        nc.vector.tensor_scalar_mul(dst, src, oh)
                    else:
                        nc.vector.scalar_tensor_tensor(
                            dst, src, oh, dst if e else base,
                            op0=ALU.mult, op1=ALU.add)

            def readback(ago):
                # agout rows = (rank, layer, ch); h-dim d = rank*CH + ch.
                # X k-tile k partition p <- d = k*128+p: ranks 2k, 2k+1.
                # One DMA per (X, k): 2 contiguous 32KB blocks each.
                agv = ago[:].rearrange(
                    "(ko r l c) m -> l r c ko m", ko=nk, r=2, l=2, c=CH)
                for r2 in range(2):
                    nc.gpsimd.dma_start(
                        X1[r2 * CH:(r2 + 1) * CH, :]
                        .rearrange("c (k m) -> c k m", k=nk), agv[1, r2])
                for r2 in range(2):
                    nc.sync.dma_start(
                        X0[r2 * CH:(r2 + 1) * CH, :]
                        .rearrange("c (k m) -> c k m", k=nk), agv[0, r2])

            # ---------- sequential phase ----------
            agout_tiles = {}
            hC = gp.tile([128, 128], F32, tag="hC")   # [h0 | h1] local
            nc.vector.memset(hC[:], 0.0)

            for s in range(T):
                sc1 = max(s - 1, 0)   # L1 lane step index (dummy at s=0)
                # tables for this step: issue first (overlaps the AG)
                tabs = sq.tile([128, 4 * CH], F32, tag="tabs")
                nc.sync.dma_start(tabs[0:B, :], gi0_tab[s * B:(s + 1) * B, :])
                nc.sync.dma_start(
                    tabs[B:128, :], gi0_tab[(T - 1 - s) * B:(T - s) * B, :])
                if s > 0:
                    ago = agout_tiles.pop(s - 1)
                    readback(ago)
                    nc.gpsimd.dma_start(X0[116:121, 3 * M:4 * M], d_ohT[s - 1])
                    nc.gpsimd.dma_start(X1[116:121, 3 * M:4 * M], d_ohT[s - 1])

                pA = psA.tile([128, 1024], F32, tag="pmA")
                pB = psB.tile([128, 1024], F32, tag="pmB")
                pC = psC.tile([128, 512], F32, tag="pmC")
                mm_seq(pA, pB, pC, X0, X1)

                # selects -> rzC [L0 | L1], nCs [inn1 | hn0 | hn1]
                rzC = gp.tile([128, 256], F32, tag="rzC")
                nCs = gp.tile([128, 192], F32, tag="nCs")
                sel5(nCs[:, 128:192], pC, sc1, 0, CH)                  # hn1
                sel5(rzC[:, 0:128], pA, s, 0, 128, base=tabs[:, 0:128])  # rz0
                sel5(nCs[:, 64:128], pA, s, NRZ, CH,
                     base=tabs[:, 192:256])                            # hn0
                sel5(rzC[:, 128:256], pB, sc1, 0, 128)                 # rz1
                sel5(nCs[:, 0:CH], pB, sc1, NRZ, CH)                   # inn1

                # combined gates ([128,128] ops, lanes L0|L1) on DVE
                act = gp.tile([128, 256], F32, tag="act")
                nc.scalar.activation(act[:], rzC[:], AF.Sigmoid)
                lrz = act[:].rearrange("p (l rz c) -> p l rz c", l=2, rz=2)
                r_ap, z_ap = lrz[:, :, 0, :], lrz[:, :, 1, :]
                rhn = gp.tile([128, 128], F32, tag="rhn")
                rhn2 = rhn[:].rearrange("p (l c) -> p l c", l=2)
                nc.vector.tensor_tensor(
                    rhn2, r_ap,
                    nCs[:, 64:192].rearrange("p (l c) -> p l c", l=2),
                    op=ALU.mult)
                nc.vector.tensor_tensor(rhn[:, 0:CH], rhn[:, 0:CH],
                                        tabs[:, 128:192], op=ALU.add)
                nc.vector.tensor_tensor(rhn[:, CH:128], rhn[:, CH:128],
                                        nCs[:, 0:CH], op=ALU.add)
                nt = gp.tile([128, 128], F32, tag="nt")
                nc.scalar.activation(nt[:], rhn[:], AF.Tanh)
                tt_ = gp.tile([128, 128], F32, tag="tt_")
                nc.vector.tensor_tensor(tt_[:], nt[:], hC[:], op=ALU.subtract)
                ut = gp.tile([128, 128], F32, tag="ut")
                nc.vector.tensor_tensor(
                    ut[:].rearrange("p (l c) -> p l c", l=2), z_ap,
                    tt_[:].rearrange("p (l c) -> p l c", l=2), op=ALU.mult)
                nc.vector.tensor_tensor(tt_[:], tt_[:], ut[:],
                                        op=ALU.subtract)
                hN = gp.tile([128, 128], F32, tag="hC")
                nc.vector.scalar_tensor_tensor(
                    hN[:, 0:CH], tt_[:, 0:CH], mT_sb[:, s:s + 1],
                    hC[:, 0:CH], op0=ALU.mult, op1=ALU.add)
                nc.vector.scalar_tensor_tensor(
                    hN[:, CH:128], tt_[:, CH:128], mT_sb[:, sc1:sc1 + 1],
                    hC[:, CH:128], op0=ALU.mult, op1=ALU.add)
                if s > 0:
                    nc.gpsimd.dma_start(d_outs[s - 1], hN[:, CH:128])
                hC = hN

                # transpose both halves at once -> agin
                pT = psC.tile([128, 128], F32, tag="pT")
                nc.tensor.transpose(pT[:], hN[:], ident_sb[:])
                ttile = gp.tile([128, 128], F32R, tag="ttile")
                nc.scalar.copy(ttile[:], pT[:])
                agin = dr.tile([M, 128], F32R, tag="agin")
                nc.sync.dma_start(agin[:], ttile[:])

                agout = dr.tile([NCORES * M, 128], F32R, tag="agout")
                agout_tiles[s] = agout
                nc.gpsimd.collective_compute(
                    "AllGather", ALU.bypass,
                    replica_groups=[list(range(NCORES))],
                    ins=[agin.opt()], outs=[agout.opt()])

            # ---------- epilogue: L1(T-1) ----------
            ago = agout_tiles.pop(T - 1)
            readback(ago)
            nc.sync.dma_start(X0[116:121, 3 * M:4 * M], d_ohT[T - 1])
            nc.sync.dma_start(X1[116:121, 3 * M:4 * M], d_ohT[T - 1])
            pA = psA.tile([128, 1024], F32, tag="pmA")
            pB = psB.tile([128, 1024], F32, tag="pmB")
            pC = psC.tile([128, 512], F32, tag="pmC")
            mm_seq(pA, pB, pC, X0, X1)
            se = T - 1
            rz1 = gp.tile([128, 128], F32, tag="rzC")
            sel5(rz1[:], pB, se, 0, 128)
            in1 = gp.tile([128, CH], F32, tag="in1e")
            sel5(in1[:], pB, se, NRZ, CH)
            hn1 = gp.tile([128, CH], F32, tag="hn1e")
            sel5(hn1[:], pC, se, 0, CH)
            acte = gp.tile([128, 128], F32, tag="acte")
            nc.scalar.activation(acte[:], rz1[:], AF.Sigmoid)
            rhne = gp.tile([128, CH], F32, tag="rhne")
            nc.gpsimd.tensor_mul(rhne[:], acte[:, 0:CH], hn1[:])
            nc.gpsimd.tensor_add(rhne[:], rhne[:], in1[:])
            nte = gp.tile([128, CH], F32, tag="nte")
            nc.scalar.activation(nte[:], rhne[:], AF.Tanh)
            tte = gp.tile([128, CH], F32, tag="tte")
            nc.gpsimd.tensor_tensor(tte[:], nte[:], hC[:, CH:128], op=ALU.subtract)
            ute = gp.tile([128, CH], F32, tag="ute")
            nc.gpsimd.tensor_mul(ute[:], acte[:, CH:128], tte[:])
            nc.gpsimd.tensor_tensor(tte[:], tte[:], ute[:], op=ALU.subtract)
            h1e = gp.tile([128, CH], F32, tag="h1e")
            nc.vector.scalar_tensor_tensor(
                h1e[:], tte[:], mT_sb[:, se:se + 1], hC[:, CH:128],
                op0=ALU.mult, op1=ALU.add)
            nc.sync.dma_start(d_outs[T - 1], h1e[:])
            nc.sync.dma_start(d_hfin[0], hC[:, 0:CH])
            nc.sync.dma_start(d_hfin[1], h1e[:])
    return nc


_CACHE = {}


def kernel(tokens, emb, W_ih0, W_hh0, b_ih0, b_hh0, W_ihL, W_hhL, b_ihL, b_hhL,
           _trace=False):
    tokens = np.asarray(tokens)
    emb = np.asarray(emb, np.float32)
    T = tokens.shape[0]

    shared, per_core = _prep(tokens, emb,
                             np.asarray(W_ih0, np.float32),
                             np.asarray(W_hh0, np.float32),
                             np.asarray(b_ih0, np.float32),
                             np.asarray(b_hh0, np.float32),
                             np.asarray(W_ihL, np.float32),
                             np.asarray(W_hhL, np.float32),
                             np.asarray(b_ihL, np.float32),
                             np.asarray(b_hhL, np.float32))

    if T not in _CACHE:
        nc = bacc.Bacc(None, num_devices=NCORES)
        build_program(nc, T)
        nc.finalize()
        _CACHE[T] = nc
    nc = _CACHE[T]

    in_maps = [{**shared, **per_core[c]} for c in range(NCORES)]
    res = run_bass_kernel_spmd(nc, in_maps, list(range(NCORES)), trace=_trace)

    outs = [res.results[c]["outs"] for c in range(NCORES)]   # [T,128,CH]
    hfin = [res.results[c]["hfin"] for c in range(NCORES)]   # [2,128,CH]

    outs_full = np.concatenate(outs, axis=2)                 # [T,128,512]
    outs_f = outs_full[:, 0:B, :H]                           # [T,B,H]
    outs_b = outs_full[:, B:128, :H]
    output = (outs_f[::-1] + outs_b) * 0.5

    hf = np.concatenate(hfin, axis=2)                        # [2,128,512]
    h0 = np.stack([hf[0, 0:B, :H], hf[1, 0:B, :H],
                   hf[0, B:128, :H], hf[1, B:128, :H]])      # [4,B,H]
    if _trace:
        return (output.astype(np.float32), h0.astype(np.float32)), res
    return output.astype(np.float32), h0.astype(np.float32)
